# revision 10
# baseline (speedup 1.0000x reference)
"""MoE ExpertLayer kernel for Trainium2 (8 NeuronCores, data-parallel over tokens).

Reference computation (B=4, S=2048, D=1024, E=8):
    logits  = x @ W_router.T + b_router          # [B,S,E]
    probs   = softmax(logits, axis=-1)
    y_e     = x @ W_experts[e].T + b_experts[e]  # all experts, dense
    out     = sum_e probs[..., e] * y_e          # [B,S,D]

Sharding: data-parallel over the flattened token axis (8192 tokens -> 1024
tokens per core). Every core receives the full (transposed) expert weights and
computes its token shard end-to-end; no collectives are needed.

Per-core dataflow (two-phase; microbenched DR/bf16 matmul stream = 216ns per
N=512 instruction regardless of fp8/bf16 mode mixing, so the schedule aims at
an uninterrupted PE instruction stream):
  - Hybrid precision: per (expert, token-tile, col-half) the K=1024
    contraction runs as 1 fp8e4m3 DoubleRow matmul (first 256 of K, full 2x
    rate) + 6 bf16 matmuls, all accumulating in one fp32 PSUM group. 14
    instructions per (e, tt) instead of 16 all-bf16. Scale-relative absmax
    error ~1.85e-2 (gate 2e-2), dominated by fp8 quantization of x and W;
    bit-stable across runs (deterministic schedule + fixed-seed inputs).
  - Expert weights/biases are pre-scaled by 256 on the host so fp8 weights
    sit in e4m3's normal range; the host divides the output by 256 after.
  - Phase R(th): router logits for one token half accumulate in the first 8
    partitions of a rotating PSUM bank as the x dt-tiles arrive from DMA;
    z = Exp(logits + b_router) on the ACT copy out of PSUM (no max
    subtraction: |logits| <= ~2.6). z transposes token-major via DVE 32x32
    block transposes; probs = z * (1/sum z) via DVE reduce/reciprocal. The
    expert-bias fold acc[t,f] = sum_e z[t,e] b_e[f] / Z is a K=8 matmul with
    z.T stationary; the 1/Z normalization rides the PSUM->acc copy on the
    otherwise-idle ACT engine.
  - Phase E(e, tts): per token tile, [DR pe0, DR pe1, 6x (bf16 pe0, bf16
    pe1)] back-to-back; the combine acc += psum * probs[:,e] is one fused
    DVE scalar_tensor_tensor per half-tile. PSUM: pe0/pe1 x3 bufs + bias x2
    = 8 banks; triple buffering keeps group-opens ahead of combine drains.
  - Order: warmup (PE p-state ramp over a memset tile while DMA delivers),
    R(th0), E(0, tt0-3), R(th1), E(0, tt4-7), then E(1..7, all tts) as pure
    streams with weights double-buffered two experts ahead.
  - Head DMA: all gating tensors + DR operands (x8 th0, W8[0]) land first
    (~0.6MB), then x th0 dt-tiles interleaved with W[0] k-chunks across both
    HWDGE rings; th1/W[1] follow. Expert 0 starts ~9us in; experts 1..7 are
    DMA-independent (1.8MB/expert vs 24us compute).
  - The final expert writes fp16 half-tiles which stream to DRAM as they
    finish (fp16 rounding invisible at this error scale). Fixed costs: ~3us
    engine preamble + ~4.5us end-of-NEFF barrier cascade.
"""

import os
import sys

for _p in ("/opt/trn_rl_repo", "/root/.axon_site/_ro/trn_rl_repo"):
    if os.path.isdir(_p) and _p not in sys.path:
        sys.path.insert(0, _p)

from contextlib import ExitStack

import ml_dtypes
import numpy as np

import concourse.bass as bass
import concourse.mybir as mybir
import concourse.tile as tile
from concourse import bacc
from concourse.bass import ts
from concourse.bass_utils import run_bass_kernel_spmd

B, S, D, E = 4, 2048, 1024, 8
N_CORES = 8
T = B * S // N_CORES  # tokens per core = 1024
P = 128               # partitions
TT = T // P           # token tiles per core = 8
DT = D // P           # contraction tiles = 8
FN = 512              # matmul moving free dim (one PSUM bank of fp32)
FH = D // FN          # output column halves = 2
TH = 2                # token halves per core
THT = T // TH         # 512 tokens per half

KP = 2                # fp8 k-pair (2 x 128 = 256 of K) per DR matmul
BFT = DT - KP         # bf16 k-tiles = 6
SW = 256.0            # host-side expert weight/bias scale


def build():
    """Build the per-core Bass/Tile program (identical SPMD program on all cores)."""
    bf16 = mybir.dt.bfloat16
    f8 = mybir.dt.float8e4
    f16 = mybir.dt.float16
    f32 = mybir.dt.float32
    DR = mybir.MatmulPerfMode.DoubleRow

    nc = bacc.Bacc("TRN2", target_bir_lowering=False, debug=False)

    xT_d = nc.dram_tensor("xT", [P, TH, DT, THT], bf16, kind="ExternalInput").ap()
    x8_d = nc.dram_tensor("x8", [P, TH, KP, THT], f8, kind="ExternalInput").ap()
    Wt_d = nc.dram_tensor("Wt", [E, P, BFT, D], bf16, kind="ExternalInput").ap()
    W8_d = nc.dram_tensor("W8", [E, P, KP, D], f8, kind="ExternalInput").ap()
    be_d = nc.dram_tensor("be", [E, D], bf16, kind="ExternalInput").ap()
    WrT_d = nc.dram_tensor("WrT", [P, DT, E], bf16, kind="ExternalInput").ap()
    brT_d = nc.dram_tensor("brT", [E, 16], f32, kind="ExternalInput").ap()
    out_d = nc.dram_tensor("out", [T, D], f16, kind="ExternalOutput").ap()

    with tile.TileContext(nc) as tc, ExitStack() as ctx:
        singles = ctx.enter_context(tc.tile_pool(name="singles", bufs=1))
        wpool = ctx.enter_context(tc.tile_pool(name="wpool", bufs=3))
        w8pool = ctx.enter_context(tc.tile_pool(name="w8pool", bufs=3))
        small = ctx.enter_context(tc.tile_pool(name="small", bufs=4))
        opool = ctx.enter_context(tc.tile_pool(name="opool", bufs=4))
        ppool = ctx.enter_context(tc.tile_pool(name="psum_e", bufs=3, space="PSUM"))
        pbias = ctx.enter_context(tc.tile_pool(name="psum_b", bufs=2, space="PSUM"))

        rA, rB = nc.sync, nc.scalar  # the two HWDGE rings

        WrT = singles.tile([P, DT, E], bf16)
        brT = singles.tile([E, 16], f32)
        be = singles.tile([E, D], bf16)
        xT = singles.tile([P, TH, DT, THT], bf16)
        x8 = singles.tile([P, TH, KP, THT], f8)
        w8_0 = w8pool.tile([P, KP, D], f8, tag="w8")
        w0 = wpool.tile([P, BFT, D], bf16, tag="w")

        # ---- Head DMA schedule (issue order per ring == arrival order;
        # trigger instructions cost ~0.6us on their host queue, and ring B =
        # the ACT queue, so B's up-front trigger batch is kept short — the
        # th1/expert-1 triggers are emitted mid-program, after the router-th0
        # Exp, so they don't delay it).
        # Order matches PE need-times: router x first, then DR operands,
        # then W[0] k-chunks; tiny gating tensors ride between bulk streams.
        rA.dma_start(out=xT[:, 0, 0:2], in_=xT_d[:, 0, 0:2])
        rA.dma_start(out=xT[:, 0, 4:6], in_=xT_d[:, 0, 4:6])
        rA.dma_start(out=x8[:, 0], in_=x8_d[:, 0])
        rA.dma_start(out=w0[:, 0:1], in_=Wt_d[0, :, 0:1])
        rA.dma_start(out=w0[:, 2:3], in_=Wt_d[0, :, 2:3])
        rA.dma_start(out=w0[:, 4:5], in_=Wt_d[0, :, 4:5])
        rB.dma_start(out=WrT, in_=WrT_d)
        rB.dma_start(out=xT[:, 0, 2:4], in_=xT_d[:, 0, 2:4])
        rB.dma_start(out=xT[:, 0, 6:8], in_=xT_d[:, 0, 6:8])
        rB.dma_start(out=w8_0, in_=W8_d[0])
        rB.dma_start(out=brT, in_=brT_d)
        rB.dma_start(out=w0[:, 1:2], in_=Wt_d[0, :, 1:2])
        rB.dma_start(out=w0[:, 3:4], in_=Wt_d[0, :, 3:4])
        rB.dma_start(out=w0[:, 5:6], in_=Wt_d[0, :, 5:6])
        rB.dma_start(out=be, in_=be_d)

        # ---- A short PE warm-up over a memset tile starts the p-state ramp
        # during the preamble-to-first-data window (~7-8.6us).
        scratch = singles.tile([P, FN], bf16)
        nc.gpsimd.memset(scratch, 0.0)
        pwarm = ppool.tile([P, FN], f32, tag="pe0")
        for _ in range(4):
            nc.tensor.matmul(pwarm, scratch[:, :P], scratch, start=True, stop=True)

        acc = singles.tile([P, TT, D], f32)
        probs = singles.tile([P, TT, E], f32)
        # z lives on 32 partitions (zero-padded past E) so the [8,THT]->[tok,8]
        # transpose runs as DVE 32x32 block transposes (no PSUM/PE time).
        zTp = singles.tile([32, TH, THT], f32)
        nc.vector.memset(zTp, 0.0)
        zTb = singles.tile([E, TH, THT], bf16)
        pTs = singles.tile([P, TT, 32], f32)

        out_dst = out_d.rearrange("(tt p) f -> p tt f", p=P)
        wrings = [rA, rB]

        def router_logits(th):
            # logits accumulate in the first 8 partitions of a pbias-ring
            # bank (free during the router window, and this keeps the expert
            # pe0/pe1 rotation rings uncontended)
            prt = pbias.tile([P, FN], f32, tag="pb")
            pr = prt[:E, :]
            for dt_ in range(DT):
                nc.tensor.matmul(
                    pr, WrT[:, dt_, :], xT[:, th, dt_, :],
                    start=(dt_ == 0), stop=(dt_ == DT - 1),
                )
            nc.scalar.activation(
                out=zTp[:E, th], in_=pr,
                func=mybir.ActivationFunctionType.Exp, bias=brT[:, 0:1], scale=1.0,
            )
            nc.vector.tensor_copy(zTb[:, th], zTp[:E, th])

        recs = {}

        def router_probs(tt):
            # DVE-only softmax tail: transpose z token-major, normalize.
            th = tt // (TT // TH)
            ti = tt % (TT // TH)
            for j in range(P // 32):
                nc.vector.transpose(
                    out=pTs[j * 32 : (j + 1) * 32, tt, :],
                    in_=zTp[:, th, ti * P + j * 32 : ti * P + (j + 1) * 32],
                )
            pT = pTs[:, tt, :E]
            ssum = small.tile([P, 1], f32, tag="ssum")
            nc.vector.reduce_sum(out=ssum, in_=pT, axis=mybir.AxisListType.X)
            rec = small.tile([P, 1], f32, tag="rec")
            nc.vector.reciprocal(rec, ssum)
            nc.vector.tensor_scalar_mul(probs[:, tt, :], pT, rec)
            recs[tt] = rec

        def bias_fold(tt):
            # acc[t, f] = sum_e z[t, e] * b_e[f] / sum(z): K=8 matmul with
            # z.T stationary; the 1/Z normalization rides the PSUM->acc copy
            # on the ACT engine. Must be emitted before the combines of this
            # tt (they read-modify acc).
            th = tt // (TT // TH)
            tok = ts(tt % (TT // TH), P)
            rec = recs.pop(tt)
            for fh in range(FH):
                pb = pbias.tile([P, FN], f32, tag="pb")
                nc.tensor.matmul(
                    pb, zTb[:, th, tok], be[:, ts(fh, FN)],
                    start=True, stop=True,
                )
                if fh == 0:
                    nc.scalar.activation(
                        out=acc[:, tt, ts(fh, FN)], in_=pb,
                        func=mybir.ActivationFunctionType.Identity, scale=rec,
                    )
                else:
                    # split the PSUM->acc drains across ACT and DVE so the
                    # copy chain doesn't serialize on one engine
                    nc.vector.tensor_scalar_mul(acc[:, tt, ts(fh, FN)], pb, rec)

        def expert_dr(w8, tt):
            # open the (pe0, pe1) PSUM group for tt with the fp8 DR pair
            th = tt // (TT // TH)
            tok = ts(tt % (TT // TH), P)
            pe0 = ppool.tile([P, FN], f32, tag="pe0")
            pe1 = ppool.tile([P, FN], f32, tag="pe1")
            lhs8 = x8[:, th, :, tok]
            nc.tensor.matmul(
                pe0, lhs8, w8[:, :, 0:FN], start=True, stop=False,
                perf_mode=mybir.MatmulPerfMode.DoubleRow,
            )
            nc.tensor.matmul(
                pe1, lhs8, w8[:, :, FN : 2 * FN], start=True, stop=False,
                perf_mode=mybir.MatmulPerfMode.DoubleRow,
            )
            return pe0, pe1

        def expert_bf16(w, tt, pes):
            pe0, pe1 = pes
            th = tt // (TT // TH)
            tok = ts(tt % (TT // TH), P)
            for dt_ in range(BFT):
                lhsT = xT[:, th, KP + dt_, tok]
                sp = dt_ == BFT - 1
                nc.tensor.matmul(pe0, lhsT, w[:, dt_, 0:FN], start=False, stop=sp)
                nc.tensor.matmul(
                    pe1, lhsT, w[:, dt_, FN : 2 * FN], start=False, stop=sp
                )

        def expert_combine(e, tt, pes):
            for fh, pe_ in ((0, pes[0]), (1, pes[1])):
                if e == E - 1:
                    # final expert: write the finished fp16 half-tile and
                    # stream it out so stores overlap remaining compute
                    o16 = opool.tile([P, FN], f16, tag="o16")
                    nc.vector.scalar_tensor_tensor(
                        out=o16, in0=pe_, scalar=probs[:, tt, e : e + 1],
                        in1=acc[:, tt, ts(fh, FN)],
                        op0=mybir.AluOpType.mult, op1=mybir.AluOpType.add,
                    )
                    wrings[fh].dma_start(out=out_dst[:, tt, ts(fh, FN)], in_=o16)
                else:
                    # acc = psum * probs[:, e] + acc  (one fused DVE op)
                    nc.vector.scalar_tensor_tensor(
                        out=acc[:, tt, ts(fh, FN)], in0=pe_,
                        scalar=probs[:, tt, e : e + 1],
                        in1=acc[:, tt, ts(fh, FN)],
                        op0=mybir.AluOpType.mult, op1=mybir.AluOpType.add,
                    )

        def expert_block(e, w, w8, tts, pf=None):
            for tt in tts:
                if pf is not None and tt < len(pf):
                    pf[tt]()
                pes = expert_dr(w8, tt)
                expert_bf16(w, tt, pes)
                expert_combine(e, tt, pes)

        def prefetch_actions(e, w, w8):
            # one ~0.3MB weight chunk per token-tile block: spreads the DMA
            # bursts so SBUF write pressure doesn't stall the PE read path
            acts = [lambda w8=w8, e=e: rA.dma_start(out=w8, in_=W8_d[e])]
            for k in range(BFT):
                r = rA if k % 2 == 0 else rB
                acts.append(
                    lambda w=w, e=e, k=k, r=r: r.dma_start(
                        out=w[:, k : k + 1], in_=Wt_d[e, :, k : k + 1]
                    )
                )
            return acts

        # ---- Phase order per half: R(th); DR-open tt/tt+1 (x8/w8 land
        # first); their bf16 streams (w k-chunk paced); bias folds (by then
        # the ACT Exp + DVE softmax tail are long done -> no PE stall);
        # combines; then the remaining two tts as full blocks. Experts 1..7
        # are pure 14-instruction streams.
        router_logits(0)
        for tt in range(0, 4):
            router_probs(tt)
        p0 = expert_dr(w8_0, 0)
        p1 = expert_dr(w8_0, 1)
        # th1 / expert-1 loads, emitted here so ring B's up-front trigger
        # batch stays short (ring B shares the ACT queue with the Exp above).
        rA.dma_start(out=xT[:, 1, 0:4], in_=xT_d[:, 1, 0:4])
        rA.dma_start(out=xT[:, 1, 4:8], in_=xT_d[:, 1, 4:8])
        rB.dma_start(out=x8[:, 1], in_=x8_d[:, 1])
        w1 = wpool.tile([P, BFT, D], bf16, tag="w")
        w8_1 = w8pool.tile([P, KP, D], f8, tag="w8")
        rA.dma_start(out=w8_1, in_=W8_d[1])
        rA.dma_start(out=w1[:, 0:3, :], in_=Wt_d[1, :, 0:3, :])
        rB.dma_start(out=w1[:, 3:6, :], in_=Wt_d[1, :, 3:6, :])
        expert_bf16(w0, 0, p0)
        expert_bf16(w0, 1, p1)
        bias_fold(0)
        bias_fold(1)
        expert_combine(0, 0, p0)
        expert_combine(0, 1, p1)
        for tt in (2, 3):
            pes = expert_dr(w8_0, tt)
            bias_fold(tt)
            expert_bf16(w0, tt, pes)
            expert_combine(0, tt, pes)

        router_logits(1)
        for tt in range(4, TT):
            router_probs(tt)
        for tt in range(4, TT):
            pes = expert_dr(w8_0, tt)
            bias_fold(tt)
            expert_bf16(w0, tt, pes)
            expert_combine(0, tt, pes)

        ws = {1: (w1, w8_1)}
        for e in range(1, E):
            pf = None
            if e + 1 < E:
                wn = wpool.tile([P, BFT, D], bf16, tag="w")
                w8n = w8pool.tile([P, KP, D], f8, tag="w8")
                ws[e + 1] = (wn, w8n)
                pf = prefetch_actions(e + 1, wn, w8n)
            w, w8 = ws.pop(e)
            expert_block(e, w, w8, range(TT), pf=pf)

    nc.compile()
    return nc


def prep_inputs(x, W_experts, b_experts, W_router, b_router):
    """Host-side marshalling: shard tokens, transpose so the contraction dim
    is DMA-contiguous onto SBUF partitions, cast/scale to compute dtypes."""
    bf = ml_dtypes.bfloat16
    f8 = ml_dtypes.float8_e4m3fn
    x = np.asarray(x, dtype=np.float32).reshape(B * S, D)
    WeT = np.asarray(W_experts, dtype=np.float32).transpose(0, 2, 1) * SW
    # bf16 k-tiles KP..DT: [E, D_in, D_out] -> [E, P, BFT, D_out]
    Wt = np.ascontiguousarray(
        WeT[:, KP * P :, :].reshape(E, BFT, P, D).transpose(0, 2, 1, 3)
    ).astype(bf)
    W8 = np.ascontiguousarray(
        WeT[:, : KP * P, :].reshape(E, KP, P, D).transpose(0, 2, 1, 3)
    ).astype(f8)
    WrT = np.ascontiguousarray(
        np.asarray(W_router, dtype=np.float32).T.reshape(DT, P, E).transpose(1, 0, 2)
    ).astype(bf)
    be = (np.asarray(b_experts, dtype=np.float32) * SW).astype(bf)
    brT = np.zeros((E, 16), np.float32)
    brT[:, 0] = np.asarray(b_router, dtype=np.float32)
    common = {"Wt": Wt, "W8": W8, "be": be, "WrT": WrT, "brT": brT}
    in_maps = []
    for c in range(N_CORES):
        xs = x[c * T : (c + 1) * T, :].T  # [D, T]
        xTc = np.ascontiguousarray(
            xs.reshape(DT, P, TH, THT).transpose(1, 2, 0, 3)  # [P, TH, DT, THT]
        ).astype(bf)
        x8c = np.ascontiguousarray(
            xs[: KP * P].reshape(KP, P, TH, THT).transpose(1, 2, 0, 3)
        ).astype(f8)
        in_maps.append({"xT": xTc, "x8": x8c, **common})
    return in_maps


def finalize(res):
    """Gather per-core fp16 outputs, un-scale, return [B, S, D] fp32."""
    out = np.concatenate(
        [np.asarray(res.results[c]["out"]).astype(np.float32) for c in range(N_CORES)],
        axis=0,
    )
    out *= 1.0 / SW
    return out.reshape(B, S, D)


_BUILT = {}


def get_built():
    if "nc" not in _BUILT:
        _BUILT["nc"] = build()
    return _BUILT["nc"]


def wait_device_ready(max_tries=8, sleep_s=20):
    """Poke the axon-tunneled devices until they respond. A crashed prior
    process can leave the remote exec unit wedged for a minute or two;
    the terminal recycles it on subsequent connection attempts."""
    import time

    import jax
    import jax.numpy as jnp

    for attempt in range(max_tries):
        try:
            devs = jax.devices()
            for d in devs[:1]:
                a = jax.device_put(jnp.ones((2, 2)), d)
                np.asarray(a)
            return True
        except Exception as exc:  # noqa: BLE001
            if attempt == max_tries - 1:
                raise
            print(f"device not ready (attempt {attempt + 1}): {exc}; retrying")
            time.sleep(sleep_s)
    return False


def run_spmd(in_maps, **kwargs):
    nc = get_built()
    wait_device_ready()
    try:
        return run_bass_kernel_spmd(
            nc, in_maps, core_ids=list(range(N_CORES)), **kwargs
        )
    except Exception as exc:  # noqa: BLE001
        print(f"run_bass_kernel_spmd failed ({exc}); retrying once after re-poke")
        wait_device_ready()
        return run_bass_kernel_spmd(
            nc, in_maps, core_ids=list(range(N_CORES)), **kwargs
        )


def kernel(x, W_experts, b_experts, W_router, b_router):
    in_maps = prep_inputs(x, W_experts, b_experts, W_router, b_router)
    res = run_spmd(in_maps)
    return finalize(res)


# revision 11
# speedup vs baseline: 1.0103x; 1.0103x over previous
"""MoE ExpertLayer kernel for Trainium2 (8 NeuronCores, data-parallel over tokens).

Reference computation (B=4, S=2048, D=1024, E=8):
    logits  = x @ W_router.T + b_router          # [B,S,E]
    probs   = softmax(logits, axis=-1)
    y_e     = x @ W_experts[e].T + b_experts[e]  # all experts, dense
    out     = sum_e probs[..., e] * y_e          # [B,S,D]

Sharding: data-parallel over the flattened token axis (8192 tokens -> 1024
tokens per core). Every core receives the full (transposed) expert weights and
computes its token shard end-to-end; no collectives are needed.

Per-core dataflow (two-phase; microbenched DR/bf16 matmul stream = 216ns per
N=512 instruction regardless of fp8/bf16 mode mixing, so the schedule aims at
an uninterrupted PE instruction stream):
  - Hybrid precision: per (expert, token-tile, col-half) the K=1024
    contraction runs as 1 fp8e4m3 DoubleRow matmul (first 256 of K, full 2x
    rate) + 6 bf16 matmuls, all accumulating in one fp32 PSUM group. 14
    instructions per (e, tt) instead of 16 all-bf16. Scale-relative absmax
    error ~1.85e-2 (gate 2e-2), dominated by fp8 quantization of x and W;
    bit-stable across runs (deterministic schedule + fixed-seed inputs).
  - Expert weights/biases are pre-scaled by 256 on the host so fp8 weights
    sit in e4m3's normal range; the host divides the output by 256 after.
  - Phase R(th): router logits for one token half accumulate in the first 8
    partitions of a rotating PSUM bank as the x dt-tiles arrive from DMA;
    z = Exp(logits + b_router) on the ACT copy out of PSUM (no max
    subtraction: |logits| <= ~2.6). z transposes token-major via DVE 32x32
    block transposes; probs = z * (1/sum z) via DVE reduce/reciprocal. The
    expert-bias fold acc[t,f] = sum_e z[t,e] b_e[f] / Z is a K=8 matmul with
    z.T stationary; the 1/Z normalization rides the PSUM->acc copy on the
    otherwise-idle ACT engine.
  - Phase E(e, tts): per token tile, [DR pe0, DR pe1, 6x (bf16 pe0, bf16
    pe1)] back-to-back; the combine acc += psum * probs[:,e] is one fused
    DVE scalar_tensor_tensor per half-tile. PSUM: pe0/pe1 x3 bufs + bias x2
    = 8 banks; triple buffering keeps group-opens ahead of combine drains.
  - Order: warmup (PE p-state ramp over a memset tile while DMA delivers),
    R(th0), E(0, tt0-3), R(th1), E(0, tt4-7), then E(1..7, all tts) as pure
    streams with weights double-buffered two experts ahead.
  - Head DMA: all gating tensors + DR operands (x8 th0, W8[0]) land first
    (~0.6MB), then x th0 dt-tiles interleaved with W[0] k-chunks across both
    HWDGE rings; th1/W[1] follow. Expert 0 starts ~9us in; experts 1..7 are
    DMA-independent (1.8MB/expert vs 24us compute).
  - The final expert writes fp16 half-tiles which stream to DRAM as they
    finish (fp16 rounding invisible at this error scale). Fixed costs: ~3us
    engine preamble + ~4.5us end-of-NEFF barrier cascade.
"""

import os
import sys

for _p in ("/opt/trn_rl_repo", "/root/.axon_site/_ro/trn_rl_repo"):
    if os.path.isdir(_p) and _p not in sys.path:
        sys.path.insert(0, _p)

from contextlib import ExitStack

import ml_dtypes
import numpy as np

import concourse.bass as bass
import concourse.mybir as mybir
import concourse.tile as tile
from concourse import bacc
from concourse.bass import ts
from concourse.bass_utils import run_bass_kernel_spmd

B, S, D, E = 4, 2048, 1024, 8
N_CORES = 8
T = B * S // N_CORES  # tokens per core = 1024
P = 128               # partitions
TT = T // P           # token tiles per core = 8
DT = D // P           # contraction tiles = 8
FN = 512              # matmul moving free dim (one PSUM bank of fp32)
FH = D // FN          # output column halves = 2
TH = 2                # token halves per core
THT = T // TH         # 512 tokens per half

KP = 2                # fp8 k-pair (2 x 128 = 256 of K) per DR matmul
BFT = DT - KP         # bf16 k-tiles = 6
SW = 256.0            # host-side expert weight/bias scale


def build():
    """Build the per-core Bass/Tile program (identical SPMD program on all cores)."""
    bf16 = mybir.dt.bfloat16
    f8 = mybir.dt.float8e4
    f16 = mybir.dt.float16
    f32 = mybir.dt.float32
    DR = mybir.MatmulPerfMode.DoubleRow

    nc = bacc.Bacc("TRN2", target_bir_lowering=False, debug=False)

    xT_d = nc.dram_tensor("xT", [P, TH, DT, THT], bf16, kind="ExternalInput").ap()
    x8_d = nc.dram_tensor("x8", [P, TH, KP, THT], f8, kind="ExternalInput").ap()
    Wt_d = nc.dram_tensor("Wt", [E, P, BFT, D], bf16, kind="ExternalInput").ap()
    W8_d = nc.dram_tensor("W8", [E, P, KP, D], f8, kind="ExternalInput").ap()
    be_d = nc.dram_tensor("be", [E, D], bf16, kind="ExternalInput").ap()
    WrT_d = nc.dram_tensor("WrT", [P, DT, E], bf16, kind="ExternalInput").ap()
    brT_d = nc.dram_tensor("brT", [E, 16], f32, kind="ExternalInput").ap()
    out_d = nc.dram_tensor("out", [T, D], f16, kind="ExternalOutput").ap()

    with tile.TileContext(nc) as tc, ExitStack() as ctx:
        singles = ctx.enter_context(tc.tile_pool(name="singles", bufs=1))
        wpool = ctx.enter_context(tc.tile_pool(name="wpool", bufs=3))
        w8pool = ctx.enter_context(tc.tile_pool(name="w8pool", bufs=3))
        small = ctx.enter_context(tc.tile_pool(name="small", bufs=4))
        opool = ctx.enter_context(tc.tile_pool(name="opool", bufs=4))
        ppool = ctx.enter_context(tc.tile_pool(name="psum_e", bufs=3, space="PSUM"))
        pbias = ctx.enter_context(tc.tile_pool(name="psum_b", bufs=2, space="PSUM"))

        rA, rB = nc.sync, nc.scalar  # the two HWDGE rings

        WrT = singles.tile([P, DT, E], bf16)
        brT = singles.tile([E, 16], f32)
        be = singles.tile([E, D], bf16)
        xT = singles.tile([P, TH, DT, THT], bf16)
        x8 = singles.tile([P, TH, KP, THT], f8)
        w8_0 = w8pool.tile([P, KP, D], f8, tag="w8")
        w0 = wpool.tile([P, BFT, D], bf16, tag="w")

        # ---- Head DMA schedule (issue order per ring == arrival order;
        # trigger instructions cost ~0.6us on their host queue, and ring B =
        # the ACT queue, so B's up-front trigger batch is kept short — the
        # th1/expert-1 triggers are emitted mid-program, after the router-th0
        # Exp, so they don't delay it).
        # Order matches PE need-times: router x first, then DR operands,
        # then W[0] k-chunks; tiny gating tensors ride between bulk streams.
        rA.dma_start(out=xT[:, 0, 0:2], in_=xT_d[:, 0, 0:2])
        rA.dma_start(out=xT[:, 0, 4:6], in_=xT_d[:, 0, 4:6])
        rA.dma_start(out=x8[:, 0], in_=x8_d[:, 0])
        rA.dma_start(out=w0[:, 0:1], in_=Wt_d[0, :, 0:1])
        rA.dma_start(out=w0[:, 2:3], in_=Wt_d[0, :, 2:3])
        rA.dma_start(out=w0[:, 4:5], in_=Wt_d[0, :, 4:5])
        rB.dma_start(out=WrT, in_=WrT_d)
        rB.dma_start(out=xT[:, 0, 2:4], in_=xT_d[:, 0, 2:4])
        rB.dma_start(out=xT[:, 0, 6:8], in_=xT_d[:, 0, 6:8])
        rB.dma_start(out=w8_0, in_=W8_d[0])
        rB.dma_start(out=brT, in_=brT_d)
        rB.dma_start(out=w0[:, 1:2], in_=Wt_d[0, :, 1:2])
        rB.dma_start(out=w0[:, 3:4], in_=Wt_d[0, :, 3:4])
        rB.dma_start(out=w0[:, 5:6], in_=Wt_d[0, :, 5:6])
        rB.dma_start(out=be, in_=be_d)

        # ---- A short PE warm-up over a memset tile starts the p-state ramp
        # during the preamble-to-first-data window (~7-8.6us).
        scratch = singles.tile([P, FN], bf16)
        nc.vector.memset(scratch, 0.0)
        pwarm = ppool.tile([P, FN], f32, tag="pe0")
        for _ in range(4):
            nc.tensor.matmul(pwarm, scratch[:, :P], scratch, start=True, stop=True)

        acc = singles.tile([P, TT, D], f32)
        probs = singles.tile([P, TT, E], f32)
        # z lives on 32 partitions (zero-padded past E) so the [8,THT]->[tok,8]
        # transpose runs as DVE 32x32 block transposes (no PSUM/PE time).
        zTp = singles.tile([32, TH, THT], f32)
        nc.vector.memset(zTp, 0.0)
        zTb = singles.tile([E, TH, THT], bf16)
        pTs = singles.tile([P, TT, 32], f32)

        out_dst = out_d.rearrange("(tt p) f -> p tt f", p=P)
        wrings = [rA, rB]

        def router_logits(th):
            # logits accumulate in the first 8 partitions of a pe0-ring bank
            prt = ppool.tile([P, FN], f32, tag="pe0")
            pr = prt[:E, :]
            for dt_ in range(DT):
                nc.tensor.matmul(
                    pr, WrT[:, dt_, :], xT[:, th, dt_, :],
                    start=(dt_ == 0), stop=(dt_ == DT - 1),
                )
            nc.scalar.activation(
                out=zTp[:E, th], in_=pr,
                func=mybir.ActivationFunctionType.Exp, bias=brT[:, 0:1], scale=1.0,
            )
            nc.vector.tensor_copy(zTb[:, th], zTp[:E, th])

        recs = {}

        def router_probs(tt):
            # DVE-only softmax tail: transpose z token-major, normalize.
            th = tt // (TT // TH)
            ti = tt % (TT // TH)
            for j in range(P // 32):
                nc.vector.transpose(
                    out=pTs[j * 32 : (j + 1) * 32, tt, :],
                    in_=zTp[:, th, ti * P + j * 32 : ti * P + (j + 1) * 32],
                )
            pT = pTs[:, tt, :E]
            ssum = small.tile([P, 1], f32, tag="ssum")
            nc.vector.reduce_sum(out=ssum, in_=pT, axis=mybir.AxisListType.X)
            rec = small.tile([P, 1], f32, tag="rec")
            nc.vector.reciprocal(rec, ssum)
            nc.vector.tensor_scalar_mul(probs[:, tt, :], pT, rec)
            recs[tt] = rec

        def bias_fold(tt):
            # acc[t, f] = sum_e z[t, e] * b_e[f] / sum(z): K=8 matmul with
            # z.T stationary; the 1/Z normalization rides the PSUM->acc copy
            # on the ACT engine. Must be emitted before the combines of this
            # tt (they read-modify acc).
            th = tt // (TT // TH)
            tok = ts(tt % (TT // TH), P)
            rec = recs.pop(tt)
            for fh in range(FH):
                pb = pbias.tile([P, FN], f32, tag="pb")
                nc.tensor.matmul(
                    pb, zTb[:, th, tok], be[:, ts(fh, FN)],
                    start=True, stop=True,
                )
                nc.scalar.activation(
                    out=acc[:, tt, ts(fh, FN)], in_=pb,
                    func=mybir.ActivationFunctionType.Identity, scale=rec,
                )

        def expert_dr(w8, tt):
            # open the (pe0, pe1) PSUM group for tt with the fp8 DR pair
            th = tt // (TT // TH)
            tok = ts(tt % (TT // TH), P)
            pe0 = ppool.tile([P, FN], f32, tag="pe0")
            pe1 = ppool.tile([P, FN], f32, tag="pe1")
            lhs8 = x8[:, th, :, tok]
            nc.tensor.matmul(
                pe0, lhs8, w8[:, :, 0:FN], start=True, stop=False,
                perf_mode=mybir.MatmulPerfMode.DoubleRow,
            )
            nc.tensor.matmul(
                pe1, lhs8, w8[:, :, FN : 2 * FN], start=True, stop=False,
                perf_mode=mybir.MatmulPerfMode.DoubleRow,
            )
            return pe0, pe1

        def expert_bf16(w, tt, pes):
            pe0, pe1 = pes
            th = tt // (TT // TH)
            tok = ts(tt % (TT // TH), P)
            for dt_ in range(BFT):
                lhsT = xT[:, th, KP + dt_, tok]
                sp = dt_ == BFT - 1
                nc.tensor.matmul(pe0, lhsT, w[:, dt_, 0:FN], start=False, stop=sp)
                nc.tensor.matmul(
                    pe1, lhsT, w[:, dt_, FN : 2 * FN], start=False, stop=sp
                )

        def expert_combine(e, tt, pes):
            for fh, pe_ in ((0, pes[0]), (1, pes[1])):
                if e == E - 1:
                    # final expert: write the finished fp16 half-tile and
                    # stream it out so stores overlap remaining compute
                    o16 = opool.tile([P, FN], f16, tag="o16")
                    nc.vector.scalar_tensor_tensor(
                        out=o16, in0=pe_, scalar=probs[:, tt, e : e + 1],
                        in1=acc[:, tt, ts(fh, FN)],
                        op0=mybir.AluOpType.mult, op1=mybir.AluOpType.add,
                    )
                    wrings[fh].dma_start(out=out_dst[:, tt, ts(fh, FN)], in_=o16)
                else:
                    # acc = psum * probs[:, e] + acc  (one fused DVE op)
                    nc.vector.scalar_tensor_tensor(
                        out=acc[:, tt, ts(fh, FN)], in0=pe_,
                        scalar=probs[:, tt, e : e + 1],
                        in1=acc[:, tt, ts(fh, FN)],
                        op0=mybir.AluOpType.mult, op1=mybir.AluOpType.add,
                    )

        def expert_block(e, w, w8, tts, pf=None):
            for tt in tts:
                if pf is not None and tt < len(pf):
                    pf[tt]()
                pes = expert_dr(w8, tt)
                expert_bf16(w, tt, pes)
                expert_combine(e, tt, pes)

        def prefetch_actions(e, w, w8):
            # one ~0.3MB weight chunk per token-tile block: spreads the DMA
            # bursts so SBUF write pressure doesn't stall the PE read path
            acts = [lambda w8=w8, e=e: rA.dma_start(out=w8, in_=W8_d[e])]
            for k in range(BFT):
                r = rA if k % 2 == 0 else rB
                acts.append(
                    lambda w=w, e=e, k=k, r=r: r.dma_start(
                        out=w[:, k : k + 1], in_=Wt_d[e, :, k : k + 1]
                    )
                )
            return acts

        # ---- Phase order per half: R(th); DR-open tt/tt+1 (x8/w8 land
        # first); their bf16 streams (w k-chunk paced); bias folds (by then
        # the ACT Exp + DVE softmax tail are long done -> no PE stall);
        # combines; then the remaining two tts as full blocks. Experts 1..7
        # are pure 14-instruction streams.
        router_logits(0)
        for tt in range(0, 4):
            router_probs(tt)
        p0 = expert_dr(w8_0, 0)
        p1 = expert_dr(w8_0, 1)
        # th1 / expert-1 loads, emitted here so ring B's up-front trigger
        # batch stays short (ring B shares the ACT queue with the Exp above).
        rA.dma_start(out=xT[:, 1, 0:4], in_=xT_d[:, 1, 0:4])
        rA.dma_start(out=xT[:, 1, 4:8], in_=xT_d[:, 1, 4:8])
        rB.dma_start(out=x8[:, 1], in_=x8_d[:, 1])
        w1 = wpool.tile([P, BFT, D], bf16, tag="w")
        w8_1 = w8pool.tile([P, KP, D], f8, tag="w8")
        rA.dma_start(out=w8_1, in_=W8_d[1])
        rA.dma_start(out=w1[:, 0:3, :], in_=Wt_d[1, :, 0:3, :])
        rB.dma_start(out=w1[:, 3:6, :], in_=Wt_d[1, :, 3:6, :])
        expert_bf16(w0, 0, p0)
        expert_bf16(w0, 1, p1)
        bias_fold(0)
        bias_fold(1)
        expert_combine(0, 0, p0)
        expert_combine(0, 1, p1)
        for tt in (2, 3):
            pes = expert_dr(w8_0, tt)
            bias_fold(tt)
            expert_bf16(w0, tt, pes)
            expert_combine(0, tt, pes)

        router_logits(1)
        for tt in range(4, TT):
            router_probs(tt)
        p4 = expert_dr(w8_0, 4)
        p5 = expert_dr(w8_0, 5)
        expert_bf16(w0, 4, p4)
        expert_bf16(w0, 5, p5)
        bias_fold(4)
        bias_fold(5)
        expert_combine(0, 4, p4)
        expert_combine(0, 5, p5)
        for tt in (6, 7):
            pes = expert_dr(w8_0, tt)
            bias_fold(tt)
            expert_bf16(w0, tt, pes)
            expert_combine(0, tt, pes)

        ws = {1: (w1, w8_1)}
        for e in range(1, E):
            pf = None
            if e + 1 < E:
                wn = wpool.tile([P, BFT, D], bf16, tag="w")
                w8n = w8pool.tile([P, KP, D], f8, tag="w8")
                ws[e + 1] = (wn, w8n)
                pf = prefetch_actions(e + 1, wn, w8n)
            w, w8 = ws.pop(e)
            expert_block(e, w, w8, range(TT), pf=pf)

    nc.compile()
    return nc


def prep_inputs(x, W_experts, b_experts, W_router, b_router):
    """Host-side marshalling: shard tokens, transpose so the contraction dim
    is DMA-contiguous onto SBUF partitions, cast/scale to compute dtypes."""
    bf = ml_dtypes.bfloat16
    f8 = ml_dtypes.float8_e4m3fn
    x = np.asarray(x, dtype=np.float32).reshape(B * S, D)
    WeT = np.asarray(W_experts, dtype=np.float32).transpose(0, 2, 1) * SW
    # bf16 k-tiles KP..DT: [E, D_in, D_out] -> [E, P, BFT, D_out]
    Wt = np.ascontiguousarray(
        WeT[:, KP * P :, :].reshape(E, BFT, P, D).transpose(0, 2, 1, 3)
    ).astype(bf)
    W8 = np.ascontiguousarray(
        WeT[:, : KP * P, :].reshape(E, KP, P, D).transpose(0, 2, 1, 3)
    ).astype(f8)
    WrT = np.ascontiguousarray(
        np.asarray(W_router, dtype=np.float32).T.reshape(DT, P, E).transpose(1, 0, 2)
    ).astype(bf)
    be = (np.asarray(b_experts, dtype=np.float32) * SW).astype(bf)
    brT = np.zeros((E, 16), np.float32)
    brT[:, 0] = np.asarray(b_router, dtype=np.float32)
    common = {"Wt": Wt, "W8": W8, "be": be, "WrT": WrT, "brT": brT}
    in_maps = []
    for c in range(N_CORES):
        xs = x[c * T : (c + 1) * T, :].T  # [D, T]
        xTc = np.ascontiguousarray(
            xs.reshape(DT, P, TH, THT).transpose(1, 2, 0, 3)  # [P, TH, DT, THT]
        ).astype(bf)
        x8c = np.ascontiguousarray(
            xs[: KP * P].reshape(KP, P, TH, THT).transpose(1, 2, 0, 3)
        ).astype(f8)
        in_maps.append({"xT": xTc, "x8": x8c, **common})
    return in_maps


def finalize(res):
    """Gather per-core fp16 outputs, un-scale, return [B, S, D] fp32."""
    out = np.concatenate(
        [np.asarray(res.results[c]["out"]).astype(np.float32) for c in range(N_CORES)],
        axis=0,
    )
    out *= 1.0 / SW
    return out.reshape(B, S, D)


_BUILT = {}


def get_built():
    if "nc" not in _BUILT:
        _BUILT["nc"] = build()
    return _BUILT["nc"]


def wait_device_ready(max_tries=8, sleep_s=20):
    """Poke the axon-tunneled devices until they respond. A crashed prior
    process can leave the remote exec unit wedged for a minute or two;
    the terminal recycles it on subsequent connection attempts."""
    import time

    import jax
    import jax.numpy as jnp

    for attempt in range(max_tries):
        try:
            devs = jax.devices()
            for d in devs[:1]:
                a = jax.device_put(jnp.ones((2, 2)), d)
                np.asarray(a)
            return True
        except Exception as exc:  # noqa: BLE001
            if attempt == max_tries - 1:
                raise
            print(f"device not ready (attempt {attempt + 1}): {exc}; retrying")
            time.sleep(sleep_s)
    return False


def run_spmd(in_maps, **kwargs):
    nc = get_built()
    wait_device_ready()
    try:
        return run_bass_kernel_spmd(
            nc, in_maps, core_ids=list(range(N_CORES)), **kwargs
        )
    except Exception as exc:  # noqa: BLE001
        print(f"run_bass_kernel_spmd failed ({exc}); retrying once after re-poke")
        wait_device_ready()
        return run_bass_kernel_spmd(
            nc, in_maps, core_ids=list(range(N_CORES)), **kwargs
        )


def kernel(x, W_experts, b_experts, W_router, b_router):
    in_maps = prep_inputs(x, W_experts, b_experts, W_router, b_router)
    res = run_spmd(in_maps)
    return finalize(res)


# revision 13
# speedup vs baseline: 1.0215x; 1.0111x over previous
"""MoE ExpertLayer kernel for Trainium2 (8 NeuronCores, data-parallel over tokens).

Reference computation (B=4, S=2048, D=1024, E=8):
    logits  = x @ W_router.T + b_router          # [B,S,E]
    probs   = softmax(logits, axis=-1)
    y_e     = x @ W_experts[e].T + b_experts[e]  # all experts, dense
    out     = sum_e probs[..., e] * y_e          # [B,S,D]

Sharding: data-parallel over the flattened token axis (8192 tokens -> 1024
tokens per core). Every core receives the full (transposed) expert weights and
computes its token shard end-to-end; no collectives are needed.

Per-core dataflow (two-phase; microbenched DR/bf16 matmul stream = 216ns per
N=512 instruction regardless of fp8/bf16 mode mixing, so the schedule aims at
an uninterrupted PE instruction stream):
  - Hybrid precision: per (expert, token-tile, col-half) the K=1024
    contraction runs as 1 fp8e4m3 DoubleRow matmul (first 256 of K, full 2x
    rate) + 6 bf16 matmuls, all accumulating in one fp32 PSUM group. 14
    instructions per (e, tt) instead of 16 all-bf16. Scale-relative absmax
    error ~1.85e-2 (gate 2e-2), dominated by fp8 quantization of x and W;
    bit-stable across runs (deterministic schedule + fixed-seed inputs).
  - Expert weights/biases are pre-scaled by 256 on the host so fp8 weights
    sit in e4m3's normal range; the host divides the output by 256 after.
  - Phase R(th): router logits for one token half accumulate in the first 8
    partitions of a rotating PSUM bank as the x dt-tiles arrive from DMA;
    z = Exp(logits + b_router) on the ACT copy out of PSUM (no max
    subtraction: |logits| <= ~2.6). z transposes token-major via DVE 32x32
    block transposes; probs = z * (1/sum z) via DVE reduce/reciprocal. The
    expert-bias fold acc[t,f] = sum_e z[t,e] b_e[f] / Z is a K=8 matmul with
    z.T stationary; the 1/Z normalization rides the PSUM->acc copy on the
    otherwise-idle ACT engine.
  - Phase E(e, tts): per token tile, [DR pe0, DR pe1, 6x (bf16 pe0, bf16
    pe1)] back-to-back; the combine acc += psum * probs[:,e] is one fused
    DVE scalar_tensor_tensor per half-tile. PSUM: pe0/pe1 x3 bufs + bias x2
    = 8 banks; triple buffering keeps group-opens ahead of combine drains.
  - Order: short warmup (PE p-state ramp over a memset tile during the
    preamble-to-first-data window), R(th0), E(0, tt0-3) with bias folds
    placed after the w-gated bf16 streams, R(th1), E(0, tt4-7), then
    E(1..7, all tts) as pure 14-instruction streams.
  - Head DMA: every engine queue runs a ~7us preamble ending in an
    all-engine barrier before the first HWDGE trigger can issue, and
    triggers cost ~0.6us of queue time each (ring B = the ACT queue, so its
    up-front batch is kept short and th1/expert-1 triggers are emitted
    mid-program). Data flows from ~8.3us at ~300GB/s aggregate (8 cores
    share HBM), so expert 0's operands (x th0 1MB + W[0] 1.75MB) pace the
    head; ring order matches PE need-times. Tiny tensors are padded
    (brT [E,16]) and placed between bulk streams -- sub-64B packets
    otherwise stall a ring ~0.7us each. Experts 1..7 prefetch one ~0.3MB
    weight chunk per token-tile block (spread, not burst, to keep SBUF
    write pressure off the PE read path).
  - The final expert writes fp16 half-tiles which stream to DRAM as they
    finish (fp16 rounding invisible at this error scale). Fixed costs: ~7us
    preamble + ~4.5us end-of-NEFF barrier cascade; measured run-to-run
    variance ~+-2us from HBM/DMA contention.
"""

import os
import sys

for _p in ("/opt/trn_rl_repo", "/root/.axon_site/_ro/trn_rl_repo"):
    if os.path.isdir(_p) and _p not in sys.path:
        sys.path.insert(0, _p)

from contextlib import ExitStack

import ml_dtypes
import numpy as np

import concourse.bass as bass
import concourse.mybir as mybir
import concourse.tile as tile
from concourse import bacc
from concourse.bass import ts
from concourse.bass_utils import run_bass_kernel_spmd

B, S, D, E = 4, 2048, 1024, 8
N_CORES = 8
T = B * S // N_CORES  # tokens per core = 1024
P = 128               # partitions
TT = T // P           # token tiles per core = 8
DT = D // P           # contraction tiles = 8
FN = 512              # matmul moving free dim (one PSUM bank of fp32)
FH = D // FN          # output column halves = 2
TH = 2                # token halves per core
THT = T // TH         # 512 tokens per half

KP = 2                # fp8 k-pair (2 x 128 = 256 of K) per DR matmul
BFT = DT - KP         # bf16 k-tiles = 6
SW = 256.0            # host-side expert weight/bias scale


def build():
    """Build the per-core Bass/Tile program (identical SPMD program on all cores)."""
    bf16 = mybir.dt.bfloat16
    f8 = mybir.dt.float8e4
    f16 = mybir.dt.float16
    f32 = mybir.dt.float32
    DR = mybir.MatmulPerfMode.DoubleRow

    nc = bacc.Bacc("TRN2", target_bir_lowering=False, debug=False)

    xT_d = nc.dram_tensor("xT", [P, TH, DT, THT], bf16, kind="ExternalInput").ap()
    x8_d = nc.dram_tensor("x8", [P, TH, KP, THT], f8, kind="ExternalInput").ap()
    Wt_d = nc.dram_tensor("Wt", [E, P, BFT, D], bf16, kind="ExternalInput").ap()
    W8_d = nc.dram_tensor("W8", [E, P, KP, D], f8, kind="ExternalInput").ap()
    be_d = nc.dram_tensor("be", [E, D], bf16, kind="ExternalInput").ap()
    WrT_d = nc.dram_tensor("WrT", [P, DT, E], bf16, kind="ExternalInput").ap()
    brT_d = nc.dram_tensor("brT", [E, 16], f32, kind="ExternalInput").ap()
    out_d = nc.dram_tensor("out", [T, D], f16, kind="ExternalOutput").ap()

    with tile.TileContext(nc) as tc, ExitStack() as ctx:
        singles = ctx.enter_context(tc.tile_pool(name="singles", bufs=1))
        wpool = ctx.enter_context(tc.tile_pool(name="wpool", bufs=4))
        w8pool = ctx.enter_context(tc.tile_pool(name="w8pool", bufs=4))
        small = ctx.enter_context(tc.tile_pool(name="small", bufs=4))
        opool = ctx.enter_context(tc.tile_pool(name="opool", bufs=4))
        ppool = ctx.enter_context(tc.tile_pool(name="psum_e", bufs=3, space="PSUM"))
        pbias = ctx.enter_context(tc.tile_pool(name="psum_b", bufs=2, space="PSUM"))

        rA, rB = nc.sync, nc.scalar  # the two HWDGE rings

        WrT = singles.tile([P, DT, E], bf16)
        brT = singles.tile([E, 16], f32)
        be = singles.tile([E, D], bf16)
        xT = singles.tile([P, TH, DT, THT], bf16)
        x8 = singles.tile([P, TH, KP, THT], f8)
        w8_0 = w8pool.tile([P, KP, D], f8, tag="w8")
        w0 = wpool.tile([P, BFT, D], bf16, tag="w")

        # ---- Head DMA schedule (issue order per ring == arrival order;
        # trigger instructions cost ~0.6us on their host queue, and ring B =
        # the ACT queue, so B's up-front trigger batch is kept short — the
        # th1/expert-1 triggers are emitted mid-program, after the router-th0
        # Exp, so they don't delay it).
        # Order matches PE need-times: router x first, then DR operands,
        # then W[0] k-chunks; tiny gating tensors ride between bulk streams.
        rA.dma_start(out=xT[:, 0, 0:2], in_=xT_d[:, 0, 0:2])
        rA.dma_start(out=xT[:, 0, 4:6], in_=xT_d[:, 0, 4:6])
        rA.dma_start(out=x8[:, 0], in_=x8_d[:, 0])
        rA.dma_start(out=w0[:, 0:1], in_=Wt_d[0, :, 0:1])
        rA.dma_start(out=w0[:, 2:3], in_=Wt_d[0, :, 2:3])
        rA.dma_start(out=w0[:, 4:5], in_=Wt_d[0, :, 4:5])
        rB.dma_start(out=WrT, in_=WrT_d)
        rB.dma_start(out=xT[:, 0, 2:4], in_=xT_d[:, 0, 2:4])
        rB.dma_start(out=xT[:, 0, 6:8], in_=xT_d[:, 0, 6:8])
        rB.dma_start(out=w8_0, in_=W8_d[0])
        rB.dma_start(out=brT, in_=brT_d)
        rB.dma_start(out=w0[:, 1:2], in_=Wt_d[0, :, 1:2])
        rB.dma_start(out=w0[:, 3:4], in_=Wt_d[0, :, 3:4])
        rB.dma_start(out=w0[:, 5:6], in_=Wt_d[0, :, 5:6])
        rB.dma_start(out=be, in_=be_d)

        # ---- A short PE warm-up over a memset tile starts the p-state ramp
        # during the preamble-to-first-data window (~7-8.6us).
        scratch = singles.tile([P, FN], bf16)
        nc.gpsimd.memset(scratch, 0.0)
        pwarm = ppool.tile([P, FN], f32, tag="pe0")
        for _ in range(4):
            nc.tensor.matmul(pwarm, scratch[:, :P], scratch, start=True, stop=True)

        acc = singles.tile([P, TT, D], f32)
        probs = singles.tile([P, TT, E], f32)
        # z lives on 32 partitions (zero-padded past E) so the [8,THT]->[tok,8]
        # transpose runs as DVE 32x32 block transposes (no PSUM/PE time).
        zTp = singles.tile([32, TH, THT], f32)
        nc.vector.memset(zTp, 0.0)
        zTb = singles.tile([E, TH, THT], bf16)
        pTs = singles.tile([P, TT, 32], f32)

        out_dst = out_d.rearrange("(tt p) f -> p tt f", p=P)
        wrings = [rA, rB]

        def router_logits(th):
            # logits accumulate in the first 8 partitions of a pe0-ring bank
            prt = ppool.tile([P, FN], f32, tag="pe0")
            pr = prt[:E, :]
            for dt_ in range(DT):
                nc.tensor.matmul(
                    pr, WrT[:, dt_, :], xT[:, th, dt_, :],
                    start=(dt_ == 0), stop=(dt_ == DT - 1),
                )
            nc.scalar.activation(
                out=zTp[:E, th], in_=pr,
                func=mybir.ActivationFunctionType.Exp, bias=brT[:, 0:1], scale=1.0,
            )
            nc.vector.tensor_copy(zTb[:, th], zTp[:E, th])

        recs = {}

        def router_probs(tt):
            # DVE-only softmax tail: transpose z token-major, normalize.
            th = tt // (TT // TH)
            ti = tt % (TT // TH)
            for j in range(P // 32):
                nc.vector.transpose(
                    out=pTs[j * 32 : (j + 1) * 32, tt, :],
                    in_=zTp[:, th, ti * P + j * 32 : ti * P + (j + 1) * 32],
                )
            pT = pTs[:, tt, :E]
            ssum = small.tile([P, 1], f32, tag="ssum")
            nc.vector.reduce_sum(out=ssum, in_=pT, axis=mybir.AxisListType.X)
            rec = small.tile([P, 1], f32, tag="rec")
            nc.vector.reciprocal(rec, ssum)
            nc.vector.tensor_scalar_mul(probs[:, tt, :], pT, rec)
            recs[tt] = rec

        def bias_fold(tt):
            # acc[t, f] = sum_e z[t, e] * b_e[f] / sum(z): K=8 matmul with
            # z.T stationary; the 1/Z normalization rides the PSUM->acc copy
            # on the ACT engine. Must be emitted before the combines of this
            # tt (they read-modify acc).
            th = tt // (TT // TH)
            tok = ts(tt % (TT // TH), P)
            rec = recs.pop(tt)
            for fh in range(FH):
                pb = pbias.tile([P, FN], f32, tag="pb")
                nc.tensor.matmul(
                    pb, zTb[:, th, tok], be[:, ts(fh, FN)],
                    start=True, stop=True,
                )
                nc.scalar.activation(
                    out=acc[:, tt, ts(fh, FN)], in_=pb,
                    func=mybir.ActivationFunctionType.Identity, scale=rec,
                )

        def expert_dr(w8, tt):
            # open the (pe0, pe1) PSUM group for tt with the fp8 DR pair
            th = tt // (TT // TH)
            tok = ts(tt % (TT // TH), P)
            pe0 = ppool.tile([P, FN], f32, tag="pe0")
            pe1 = ppool.tile([P, FN], f32, tag="pe1")
            lhs8 = x8[:, th, :, tok]
            nc.tensor.matmul(
                pe0, lhs8, w8[:, :, 0:FN], start=True, stop=False,
                perf_mode=mybir.MatmulPerfMode.DoubleRow,
            )
            nc.tensor.matmul(
                pe1, lhs8, w8[:, :, FN : 2 * FN], start=True, stop=False,
                perf_mode=mybir.MatmulPerfMode.DoubleRow,
            )
            return pe0, pe1

        def expert_bf16(w, tt, pes):
            pe0, pe1 = pes
            th = tt // (TT // TH)
            tok = ts(tt % (TT // TH), P)
            for dt_ in range(BFT):
                lhsT = xT[:, th, KP + dt_, tok]
                sp = dt_ == BFT - 1
                nc.tensor.matmul(pe0, lhsT, w[:, dt_, 0:FN], start=False, stop=sp)
                nc.tensor.matmul(
                    pe1, lhsT, w[:, dt_, FN : 2 * FN], start=False, stop=sp
                )

        def expert_combine(e, tt, pes):
            for fh, pe_ in ((0, pes[0]), (1, pes[1])):
                if e == E - 1:
                    # final expert: write the finished fp16 half-tile and
                    # stream it out so stores overlap remaining compute
                    o16 = opool.tile([P, FN], f16, tag="o16")
                    nc.vector.scalar_tensor_tensor(
                        out=o16, in0=pe_, scalar=probs[:, tt, e : e + 1],
                        in1=acc[:, tt, ts(fh, FN)],
                        op0=mybir.AluOpType.mult, op1=mybir.AluOpType.add,
                    )
                    wrings[fh].dma_start(out=out_dst[:, tt, ts(fh, FN)], in_=o16)
                else:
                    # acc = psum * probs[:, e] + acc  (one fused DVE op)
                    nc.vector.scalar_tensor_tensor(
                        out=acc[:, tt, ts(fh, FN)], in0=pe_,
                        scalar=probs[:, tt, e : e + 1],
                        in1=acc[:, tt, ts(fh, FN)],
                        op0=mybir.AluOpType.mult, op1=mybir.AluOpType.add,
                    )

        def expert_block(e, w, w8, tts, pf=None):
            for tt in tts:
                if pf is not None and tt < len(pf):
                    pf[tt]()
                pes = expert_dr(w8, tt)
                expert_bf16(w, tt, pes)
                expert_combine(e, tt, pes)

        def prefetch_actions(e, w, w8):
            # one ~0.3MB weight chunk per token-tile block: spreads the DMA
            # bursts so SBUF write pressure doesn't stall the PE read path
            acts = [lambda w8=w8, e=e: rA.dma_start(out=w8, in_=W8_d[e])]
            for k in range(BFT):
                r = rA if k % 2 == 0 else rB
                acts.append(
                    lambda w=w, e=e, k=k, r=r: r.dma_start(
                        out=w[:, k : k + 1], in_=Wt_d[e, :, k : k + 1]
                    )
                )
            return acts

        # ---- Phase order per half: R(th); DR-open tt/tt+1 (x8/w8 land
        # first); their bf16 streams (w k-chunk paced); bias folds (by then
        # the ACT Exp + DVE softmax tail are long done -> no PE stall);
        # combines; then the remaining two tts as full blocks. Experts 1..7
        # are pure 14-instruction streams.
        router_logits(0)
        for tt in range(0, 4):
            router_probs(tt)
        p0 = expert_dr(w8_0, 0)
        p1 = expert_dr(w8_0, 1)
        # th1 / expert-1 loads, emitted here so ring B's up-front trigger
        # batch stays short (ring B shares the ACT queue with the Exp above).
        rA.dma_start(out=xT[:, 1, 0:4], in_=xT_d[:, 1, 0:4])
        rA.dma_start(out=xT[:, 1, 4:8], in_=xT_d[:, 1, 4:8])
        rB.dma_start(out=x8[:, 1], in_=x8_d[:, 1])
        w1 = wpool.tile([P, BFT, D], bf16, tag="w")
        w8_1 = w8pool.tile([P, KP, D], f8, tag="w8")
        rA.dma_start(out=w8_1, in_=W8_d[1])
        rA.dma_start(out=w1[:, 0:3, :], in_=Wt_d[1, :, 0:3, :])
        rB.dma_start(out=w1[:, 3:6, :], in_=Wt_d[1, :, 3:6, :])
        expert_bf16(w0, 0, p0)
        expert_bf16(w0, 1, p1)
        bias_fold(0)
        bias_fold(1)
        expert_combine(0, 0, p0)
        expert_combine(0, 1, p1)
        for tt in (2, 3):
            pes = expert_dr(w8_0, tt)
            bias_fold(tt)
            expert_bf16(w0, tt, pes)
            expert_combine(0, tt, pes)

        router_logits(1)
        for tt in range(4, TT):
            router_probs(tt)
        p4 = expert_dr(w8_0, 4)
        p5 = expert_dr(w8_0, 5)
        expert_bf16(w0, 4, p4)
        expert_bf16(w0, 5, p5)
        bias_fold(4)
        bias_fold(5)
        expert_combine(0, 4, p4)
        expert_combine(0, 5, p5)
        for tt in (6, 7):
            pes = expert_dr(w8_0, tt)
            bias_fold(tt)
            expert_bf16(w0, tt, pes)
            expert_combine(0, tt, pes)

        ws = {1: (w1, w8_1)}
        for e in range(1, E):
            pf = None
            if e + 1 < E:
                wn = wpool.tile([P, BFT, D], bf16, tag="w")
                w8n = w8pool.tile([P, KP, D], f8, tag="w8")
                ws[e + 1] = (wn, w8n)
                pf = prefetch_actions(e + 1, wn, w8n)
            w, w8 = ws.pop(e)
            expert_block(e, w, w8, range(TT), pf=pf)

    nc.compile()
    return nc


def prep_inputs(x, W_experts, b_experts, W_router, b_router):
    """Host-side marshalling: shard tokens, transpose so the contraction dim
    is DMA-contiguous onto SBUF partitions, cast/scale to compute dtypes."""
    bf = ml_dtypes.bfloat16
    f8 = ml_dtypes.float8_e4m3fn
    x = np.asarray(x, dtype=np.float32).reshape(B * S, D)
    WeT = np.asarray(W_experts, dtype=np.float32).transpose(0, 2, 1) * SW
    # bf16 k-tiles KP..DT: [E, D_in, D_out] -> [E, P, BFT, D_out]
    Wt = np.ascontiguousarray(
        WeT[:, KP * P :, :].reshape(E, BFT, P, D).transpose(0, 2, 1, 3)
    ).astype(bf)
    W8 = np.ascontiguousarray(
        WeT[:, : KP * P, :].reshape(E, KP, P, D).transpose(0, 2, 1, 3)
    ).astype(f8)
    WrT = np.ascontiguousarray(
        np.asarray(W_router, dtype=np.float32).T.reshape(DT, P, E).transpose(1, 0, 2)
    ).astype(bf)
    be = (np.asarray(b_experts, dtype=np.float32) * SW).astype(bf)
    brT = np.zeros((E, 16), np.float32)
    brT[:, 0] = np.asarray(b_router, dtype=np.float32)
    common = {"Wt": Wt, "W8": W8, "be": be, "WrT": WrT, "brT": brT}
    in_maps = []
    for c in range(N_CORES):
        xs = x[c * T : (c + 1) * T, :].T  # [D, T]
        xTc = np.ascontiguousarray(
            xs.reshape(DT, P, TH, THT).transpose(1, 2, 0, 3)  # [P, TH, DT, THT]
        ).astype(bf)
        x8c = np.ascontiguousarray(
            xs[: KP * P].reshape(KP, P, TH, THT).transpose(1, 2, 0, 3)
        ).astype(f8)
        in_maps.append({"xT": xTc, "x8": x8c, **common})
    return in_maps


def finalize(res):
    """Gather per-core fp16 outputs, un-scale, return [B, S, D] fp32."""
    out = np.concatenate(
        [np.asarray(res.results[c]["out"]).astype(np.float32) for c in range(N_CORES)],
        axis=0,
    )
    out *= 1.0 / SW
    return out.reshape(B, S, D)


_BUILT = {}


def get_built():
    if "nc" not in _BUILT:
        _BUILT["nc"] = build()
    return _BUILT["nc"]


def wait_device_ready(max_tries=8, sleep_s=20):
    """Poke the axon-tunneled devices until they respond. A crashed prior
    process can leave the remote exec unit wedged for a minute or two;
    the terminal recycles it on subsequent connection attempts."""
    import time

    import jax
    import jax.numpy as jnp

    for attempt in range(max_tries):
        try:
            devs = jax.devices()
            for d in devs[:1]:
                a = jax.device_put(jnp.ones((2, 2)), d)
                np.asarray(a)
            return True
        except Exception as exc:  # noqa: BLE001
            if attempt == max_tries - 1:
                raise
            print(f"device not ready (attempt {attempt + 1}): {exc}; retrying")
            time.sleep(sleep_s)
    return False


def run_spmd(in_maps, **kwargs):
    nc = get_built()
    wait_device_ready()
    try:
        return run_bass_kernel_spmd(
            nc, in_maps, core_ids=list(range(N_CORES)), **kwargs
        )
    except Exception as exc:  # noqa: BLE001
        print(f"run_bass_kernel_spmd failed ({exc}); retrying once after re-poke")
        wait_device_ready()
        return run_bass_kernel_spmd(
            nc, in_maps, core_ids=list(range(N_CORES)), **kwargs
        )


def kernel(x, W_experts, b_experts, W_router, b_router):
    in_maps = prep_inputs(x, W_experts, b_experts, W_router, b_router)
    res = run_spmd(in_maps)
    return finalize(res)


# revision 18
# speedup vs baseline: 1.0358x; 1.0140x over previous
"""MoE ExpertLayer kernel for Trainium2 (8 NeuronCores, data-parallel over tokens).

Reference computation (B=4, S=2048, D=1024, E=8):
    logits  = x @ W_router.T + b_router          # [B,S,E]
    probs   = softmax(logits, axis=-1)
    y_e     = x @ W_experts[e].T + b_experts[e]  # all experts, dense
    out     = sum_e probs[..., e] * y_e          # [B,S,D]

Sharding: data-parallel over the flattened token axis (8192 tokens -> 1024
tokens per core). Every core receives the full (transposed) expert weights and
computes its token shard end-to-end; no collectives are needed.

Per-core dataflow (two-phase; microbenched DR/bf16 matmul stream = 216ns per
N=512 instruction regardless of fp8/bf16 mode mixing, so the schedule aims at
an uninterrupted PE instruction stream):
  - Hybrid precision: per (expert, token-tile, col-half) the K=1024
    contraction runs as 1 fp8e4m3 DoubleRow matmul (first 256 of K, full 2x
    rate) + 6 bf16 matmuls, all accumulating in one fp32 PSUM group. 14
    instructions per (e, tt) instead of 16 all-bf16. Scale-relative absmax
    error ~1.85e-2 (gate 2e-2), dominated by fp8 quantization of x and W;
    bit-stable across runs (deterministic schedule + fixed-seed inputs).
  - Expert weights/biases are pre-scaled by 256 on the host so fp8 weights
    sit in e4m3's normal range; the host divides the output by 256 after.
  - Phase R(th): router logits for one token half accumulate in the first 8
    partitions of a rotating PSUM bank as the x dt-tiles arrive from DMA;
    z = Exp(logits + b_router) on the ACT copy out of PSUM (no max
    subtraction: |logits| <= ~2.6). z transposes token-major via DVE 32x32
    block transposes; probs = z * (1/sum z) via DVE reduce/reciprocal. The
    expert-bias fold acc[t,f] = sum_e z[t,e] b_e[f] / Z is a K=8 matmul with
    z.T stationary; the 1/Z normalization rides the PSUM->acc copy on the
    otherwise-idle ACT engine.
  - Phase E(e, tts): per token tile, [DR pe0, DR pe1, 6x (bf16 pe0, bf16
    pe1)] back-to-back; the combine acc += psum * probs[:,e] is one fused
    DVE scalar_tensor_tensor per half-tile. PSUM: pe0/pe1 x3 bufs + bias x2
    = 8 banks; triple buffering keeps group-opens ahead of combine drains.
  - Order: short warmup (PE p-state ramp over a memset tile during the
    preamble-to-first-data window), R(th0), E(0, tt0-3) with bias folds
    placed after the w-gated bf16 streams, R(th1), E(0, tt4-7), then
    E(1..7, all tts) as pure 14-instruction streams.
  - Head DMA: every engine queue runs a ~7us preamble ending in an
    all-engine barrier before the first HWDGE trigger can issue, and
    triggers cost ~0.6us of queue time each (ring B = the ACT queue, so its
    up-front batch is kept short and th1/expert-1 triggers are emitted
    mid-program). Data flows from ~8.3us at ~300GB/s aggregate (8 cores
    share HBM), so expert 0's operands (x th0 1MB + W[0] 1.75MB) pace the
    head; ring order matches PE need-times. Tiny tensors are padded
    (brT [E,16]) and placed between bulk streams -- sub-64B packets
    otherwise stall a ring ~0.7us each. Experts 1..7 prefetch one ~0.3MB
    weight chunk per token-tile block (spread, not burst, to keep SBUF
    write pressure off the PE read path).
  - The final expert writes fp16 half-tiles which stream to DRAM as they
    finish (fp16 rounding invisible at this error scale). Fixed costs: ~7us
    preamble + ~4.5us end-of-NEFF barrier cascade; measured run-to-run
    variance ~+-2us from HBM/DMA contention.
"""

import os
import sys

for _p in ("/opt/trn_rl_repo", "/root/.axon_site/_ro/trn_rl_repo"):
    if os.path.isdir(_p) and _p not in sys.path:
        sys.path.insert(0, _p)

from contextlib import ExitStack

import ml_dtypes
import numpy as np

import concourse.bass as bass
import concourse.mybir as mybir
import concourse.tile as tile
from concourse import bacc
from concourse.bass import ts
from concourse.bass_utils import run_bass_kernel_spmd

B, S, D, E = 4, 2048, 1024, 8
N_CORES = 8
T = B * S // N_CORES  # tokens per core = 1024
P = 128               # partitions
TT = T // P           # token tiles per core = 8
DT = D // P           # contraction tiles = 8
FN = 512              # matmul moving free dim (one PSUM bank of fp32)
FH = D // FN          # output column halves = 2
TH = 2                # token halves per core
THT = T // TH         # 512 tokens per half

KP = 2                # fp8 k-pair (2 x 128 = 256 of K) per DR matmul
BFT = DT - KP         # bf16 k-tiles = 6
SW = 256.0            # host-side expert weight/bias scale


def build():
    """Build the per-core Bass/Tile program (identical SPMD program on all cores)."""
    bf16 = mybir.dt.bfloat16
    f8 = mybir.dt.float8e4
    f16 = mybir.dt.float16
    f32 = mybir.dt.float32
    DR = mybir.MatmulPerfMode.DoubleRow

    nc = bacc.Bacc("TRN2", target_bir_lowering=False, debug=False)

    xT_d = nc.dram_tensor("xT", [P, TH, DT, THT], bf16, kind="ExternalInput").ap()
    x8_d = nc.dram_tensor("x8", [P, TH, KP, THT], f8, kind="ExternalInput").ap()
    Wt_d = nc.dram_tensor("Wt", [E, P, BFT, D], bf16, kind="ExternalInput").ap()
    W8_d = nc.dram_tensor("W8", [E, P, KP, D], f8, kind="ExternalInput").ap()
    be_d = nc.dram_tensor("be", [E, D], bf16, kind="ExternalInput").ap()
    WrT_d = nc.dram_tensor("WrT", [P, DT, E], bf16, kind="ExternalInput").ap()
    brT_d = nc.dram_tensor("brT", [E, 16], f32, kind="ExternalInput").ap()
    out_d = nc.dram_tensor("out", [T, D], f16, kind="ExternalOutput").ap()

    with tile.TileContext(nc) as tc, ExitStack() as ctx:
        singles = ctx.enter_context(tc.tile_pool(name="singles", bufs=1))
        wpool = ctx.enter_context(tc.tile_pool(name="wpool", bufs=4))
        w8pool = ctx.enter_context(tc.tile_pool(name="w8pool", bufs=4))
        small = ctx.enter_context(tc.tile_pool(name="small", bufs=4))
        opool = ctx.enter_context(tc.tile_pool(name="opool", bufs=4))
        ppool = ctx.enter_context(tc.tile_pool(name="psum_e", bufs=3, space="PSUM"))
        pbias = ctx.enter_context(tc.tile_pool(name="psum_b", bufs=2, space="PSUM"))

        rA, rB = nc.sync, nc.scalar  # the two HWDGE rings

        WrT = singles.tile([P, DT, E], bf16)
        brT = singles.tile([E, 16], f32)
        be = singles.tile([E, D], bf16)
        xT = singles.tile([P, TH, DT, THT], bf16)
        x8 = singles.tile([P, TH, KP, THT], f8)
        w8_0 = w8pool.tile([P, KP, D], f8, tag="w8")
        w0 = wpool.tile([P, BFT, D], bf16, tag="w")

        # ---- Head DMA schedule (issue order per ring == arrival order;
        # trigger instructions cost ~0.6us on their host queue, and ring B =
        # the ACT queue, so B's up-front trigger batch is kept short — the
        # th1/expert-1 triggers are emitted mid-program, after the router-th0
        # Exp, so they don't delay it).
        # Order matches PE need-times: router x first, then DR operands,
        # then W[0] k-chunks; tiny gating tensors ride between bulk streams.
        rA.dma_start(out=x8[:, 0], in_=x8_d[:, 0])
        rA.dma_start(out=xT[:, 0, 0:2], in_=xT_d[:, 0, 0:2])
        rA.dma_start(out=xT[:, 0, 4:6], in_=xT_d[:, 0, 4:6])
        rA.dma_start(out=w0[:, 0:1], in_=Wt_d[0, :, 0:1])
        rA.dma_start(out=w0[:, 2:3], in_=Wt_d[0, :, 2:3])
        rA.dma_start(out=w0[:, 4:5], in_=Wt_d[0, :, 4:5])
        rB.dma_start(out=w8_0, in_=W8_d[0])
        rB.dma_start(out=WrT, in_=WrT_d)
        rB.dma_start(out=xT[:, 0, 2:4], in_=xT_d[:, 0, 2:4])
        rB.dma_start(out=xT[:, 0, 6:8], in_=xT_d[:, 0, 6:8])
        rB.dma_start(out=brT, in_=brT_d)
        rB.dma_start(out=w0[:, 1:2], in_=Wt_d[0, :, 1:2])
        rB.dma_start(out=w0[:, 3:4], in_=Wt_d[0, :, 3:4])
        rB.dma_start(out=w0[:, 5:6], in_=Wt_d[0, :, 5:6])
        rB.dma_start(out=be, in_=be_d)

        # ---- A short PE warm-up over a memset tile starts the p-state ramp
        # during the preamble-to-first-data window (~7-8.6us).
        scratch = singles.tile([P, FN], bf16)
        nc.gpsimd.memset(scratch, 0.0)
        pwarm = ppool.tile([P, FN], f32, tag="pe0")
        for _ in range(8):
            nc.tensor.matmul(
                pwarm[:, :256], scratch[:, :P], scratch[:, :256],
                start=True, stop=True,
            )

        acc = singles.tile([P, TT, D], f32)
        probs = singles.tile([P, TT, E], f32)
        # z lives on 32 partitions (zero-padded past E) so the [8,THT]->[tok,8]
        # transpose runs as DVE 32x32 block transposes (no PSUM/PE time).
        zTp = singles.tile([32, TH, THT], f32)
        nc.vector.memset(zTp, 0.0)
        zTb = singles.tile([E, TH, THT], bf16)
        pTs = singles.tile([P, TT, 32], f32)

        out_dst = out_d.rearrange("(tt p) f -> p tt f", p=P)
        wrings = [rA, rB]

        def router_logits(th):
            # logits accumulate in the first 8 partitions of a pbias-ring
            # bank (frees the expert pe0 ring so three DR groups can open
            # before the router runs)
            prt = pbias.tile([P, FN], f32, tag="pb")
            pr = prt[:E, :]
            for dt_ in range(DT):
                nc.tensor.matmul(
                    pr, WrT[:, dt_, :], xT[:, th, dt_, :],
                    start=(dt_ == 0), stop=(dt_ == DT - 1),
                )
            nc.scalar.activation(
                out=zTp[:E, th], in_=pr,
                func=mybir.ActivationFunctionType.Exp, bias=brT[:, 0:1], scale=1.0,
            )
            nc.vector.tensor_copy(zTb[:, th], zTp[:E, th])

        recs = {}

        def router_probs(tt):
            # DVE-only softmax tail: transpose z token-major, normalize.
            th = tt // (TT // TH)
            ti = tt % (TT // TH)
            for j in range(P // 32):
                nc.vector.transpose(
                    out=pTs[j * 32 : (j + 1) * 32, tt, :],
                    in_=zTp[:, th, ti * P + j * 32 : ti * P + (j + 1) * 32],
                )
            pT = pTs[:, tt, :E]
            ssum = small.tile([P, 1], f32, tag="ssum")
            nc.vector.reduce_sum(out=ssum, in_=pT, axis=mybir.AxisListType.X)
            rec = small.tile([P, 1], f32, tag="rec")
            nc.vector.reciprocal(rec, ssum)
            nc.vector.tensor_scalar_mul(probs[:, tt, :], pT, rec)
            recs[tt] = rec

        def bias_fold(tt):
            # acc[t, f] = sum_e z[t, e] * b_e[f] / sum(z): K=8 matmul with
            # z.T stationary; the 1/Z normalization rides the PSUM->acc copy
            # on the ACT engine. Must be emitted before the combines of this
            # tt (they read-modify acc).
            th = tt // (TT // TH)
            tok = ts(tt % (TT // TH), P)
            rec = recs.pop(tt)
            for fh in range(FH):
                pb = pbias.tile([P, FN], f32, tag="pb")
                nc.tensor.matmul(
                    pb, zTb[:, th, tok], be[:, ts(fh, FN)],
                    start=True, stop=True,
                )
                nc.scalar.activation(
                    out=acc[:, tt, ts(fh, FN)], in_=pb,
                    func=mybir.ActivationFunctionType.Identity, scale=rec,
                )

        def expert_dr(w8, tt):
            # open the (pe0, pe1) PSUM group for tt with the fp8 DR pair
            th = tt // (TT // TH)
            tok = ts(tt % (TT // TH), P)
            pe0 = ppool.tile([P, FN], f32, tag="pe0")
            pe1 = ppool.tile([P, FN], f32, tag="pe1")
            lhs8 = x8[:, th, :, tok]
            nc.tensor.matmul(
                pe0, lhs8, w8[:, :, 0:FN], start=True, stop=False,
                perf_mode=mybir.MatmulPerfMode.DoubleRow,
            )
            nc.tensor.matmul(
                pe1, lhs8, w8[:, :, FN : 2 * FN], start=True, stop=False,
                perf_mode=mybir.MatmulPerfMode.DoubleRow,
            )
            return pe0, pe1

        def expert_bf16(w, tt, pes):
            pe0, pe1 = pes
            th = tt // (TT // TH)
            tok = ts(tt % (TT // TH), P)
            for dt_ in range(BFT):
                lhsT = xT[:, th, KP + dt_, tok]
                sp = dt_ == BFT - 1
                nc.tensor.matmul(pe0, lhsT, w[:, dt_, 0:FN], start=False, stop=sp)
                nc.tensor.matmul(
                    pe1, lhsT, w[:, dt_, FN : 2 * FN], start=False, stop=sp
                )

        def expert_combine(e, tt, pes, fhs=(0, 1)):
            for fh, pe_ in ((0, pes[0]), (1, pes[1])):
                if fh not in fhs:
                    continue
                if e == E - 1:
                    # final expert: write the finished fp16 half-tile and
                    # stream it out so stores overlap remaining compute
                    o16 = opool.tile([P, FN], f16, tag="o16")
                    nc.vector.scalar_tensor_tensor(
                        out=o16, in0=pe_, scalar=probs[:, tt, e : e + 1],
                        in1=acc[:, tt, ts(fh, FN)],
                        op0=mybir.AluOpType.mult, op1=mybir.AluOpType.add,
                    )
                    wrings[fh].dma_start(out=out_dst[:, tt, ts(fh, FN)], in_=o16)
                else:
                    # acc = psum * probs[:, e] + acc  (one fused DVE op)
                    nc.vector.scalar_tensor_tensor(
                        out=acc[:, tt, ts(fh, FN)], in0=pe_,
                        scalar=probs[:, tt, e : e + 1],
                        in1=acc[:, tt, ts(fh, FN)],
                        op0=mybir.AluOpType.mult, op1=mybir.AluOpType.add,
                    )

        def expert_block(e, w, w8, tts, pf=None):
            for tt in tts:
                if pf is not None and tt < len(pf):
                    pf[tt]()
                pes = expert_dr(w8, tt)
                if e == E - 1 and tt == TT - 1:
                    # final block: close/drain the pe0 half first so its
                    # combine+store overlap the pe1 stream (shorter tail)
                    th = tt // (TT // TH)
                    tok = ts(tt % (TT // TH), P)
                    for dt_ in range(BFT):
                        nc.tensor.matmul(
                            pes[0], xT[:, th, KP + dt_, tok], w[:, dt_, 0:FN],
                            start=False, stop=(dt_ == BFT - 1),
                        )
                    expert_combine(e, tt, pes, fhs=(0,))
                    for dt_ in range(BFT):
                        nc.tensor.matmul(
                            pes[1], xT[:, th, KP + dt_, tok],
                            w[:, dt_, FN : 2 * FN],
                            start=False, stop=(dt_ == BFT - 1),
                        )
                    expert_combine(e, tt, pes, fhs=(1,))
                else:
                    expert_bf16(w, tt, pes)
                    expert_combine(e, tt, pes)

        def prefetch_actions(e, w, w8):
            # one ~0.3MB weight chunk per token-tile block: spreads the DMA
            # bursts so SBUF write pressure doesn't stall the PE read path
            acts = [lambda w8=w8, e=e: rA.dma_start(out=w8, in_=W8_d[e])]
            for k in range(BFT):
                r = rA if k % 2 == 0 else rB
                acts.append(
                    lambda w=w, e=e, k=k, r=r: r.dma_start(
                        out=w[:, k : k + 1], in_=Wt_d[e, :, k : k + 1]
                    )
                )
            return acts

        # ---- Phase order per half: R(th); DR-open tt/tt+1 (x8/w8 land
        # first); their bf16 streams (w k-chunk paced); bias folds (by then
        # the ACT Exp + DVE softmax tail are long done -> no PE stall);
        # combines; then the remaining two tts as full blocks. Experts 1..7
        # are pure 14-instruction streams.
        p0 = expert_dr(w8_0, 0)
        p1 = expert_dr(w8_0, 1)
        p2 = expert_dr(w8_0, 2)
        router_logits(0)
        for tt in range(0, 4):
            router_probs(tt)
        # th1 / expert-1 loads, emitted here so ring B's up-front trigger
        # batch stays short (ring B shares the ACT queue with the Exp above).
        rA.dma_start(out=xT[:, 1, 0:4], in_=xT_d[:, 1, 0:4])
        rA.dma_start(out=xT[:, 1, 4:8], in_=xT_d[:, 1, 4:8])
        rB.dma_start(out=x8[:, 1], in_=x8_d[:, 1])
        w1 = wpool.tile([P, BFT, D], bf16, tag="w")
        w8_1 = w8pool.tile([P, KP, D], f8, tag="w8")
        rA.dma_start(out=w8_1, in_=W8_d[1])
        rA.dma_start(out=w1[:, 0:3, :], in_=Wt_d[1, :, 0:3, :])
        rB.dma_start(out=w1[:, 3:6, :], in_=Wt_d[1, :, 3:6, :])
        expert_bf16(w0, 0, p0)
        bias_fold(0)
        expert_combine(0, 0, p0)
        expert_bf16(w0, 1, p1)
        bias_fold(1)
        expert_combine(0, 1, p1)
        p3 = expert_dr(w8_0, 3)
        expert_bf16(w0, 2, p2)
        bias_fold(2)
        expert_combine(0, 2, p2)
        # R(th1) here: x th1 has arrived by now, and running it an expert
        # block early lets the softmax tail + bias ACT copies drain during
        # tt3 instead of stalling the th1 blocks and expert 1's start
        router_logits(1)
        for tt in range(4, TT):
            router_probs(tt)
        expert_bf16(w0, 3, p3)
        bias_fold(3)
        expert_combine(0, 3, p3)
        for tt in range(4, TT):
            pes = expert_dr(w8_0, tt)
            bias_fold(tt)
            expert_bf16(w0, tt, pes)
            expert_combine(0, tt, pes)

        ws = {1: (w1, w8_1)}
        for e in range(1, E):
            pf = None
            if e + 1 < E:
                wn = wpool.tile([P, BFT, D], bf16, tag="w")
                w8n = w8pool.tile([P, KP, D], f8, tag="w8")
                ws[e + 1] = (wn, w8n)
                pf = prefetch_actions(e + 1, wn, w8n)
            w, w8 = ws.pop(e)
            expert_block(e, w, w8, range(TT), pf=pf)

    nc.compile()
    return nc


def prep_inputs(x, W_experts, b_experts, W_router, b_router):
    """Host-side marshalling: shard tokens, transpose so the contraction dim
    is DMA-contiguous onto SBUF partitions, cast/scale to compute dtypes."""
    bf = ml_dtypes.bfloat16
    f8 = ml_dtypes.float8_e4m3fn
    x = np.asarray(x, dtype=np.float32).reshape(B * S, D)
    WeT = np.asarray(W_experts, dtype=np.float32).transpose(0, 2, 1) * SW
    # bf16 k-tiles KP..DT: [E, D_in, D_out] -> [E, P, BFT, D_out]
    Wt = np.ascontiguousarray(
        WeT[:, KP * P :, :].reshape(E, BFT, P, D).transpose(0, 2, 1, 3)
    ).astype(bf)
    W8 = np.ascontiguousarray(
        WeT[:, : KP * P, :].reshape(E, KP, P, D).transpose(0, 2, 1, 3)
    ).astype(f8)
    WrT = np.ascontiguousarray(
        np.asarray(W_router, dtype=np.float32).T.reshape(DT, P, E).transpose(1, 0, 2)
    ).astype(bf)
    be = (np.asarray(b_experts, dtype=np.float32) * SW).astype(bf)
    brT = np.zeros((E, 16), np.float32)
    brT[:, 0] = np.asarray(b_router, dtype=np.float32)
    in_maps = []
    for c in range(N_CORES):
        # rotate the expert axis per core so the 8 cores' head-phase weight
        # reads hit different HBM regions instead of all pulling W[0] at
        # once; the math is order-invariant (probs/bias rotate with it)
        rot = [(j + c) % E for j in range(E)]
        xs = x[c * T : (c + 1) * T, :].T  # [D, T]
        xTc = np.ascontiguousarray(
            xs.reshape(DT, P, TH, THT).transpose(1, 2, 0, 3)  # [P, TH, DT, THT]
        ).astype(bf)
        x8c = np.ascontiguousarray(
            xs[: KP * P].reshape(KP, P, TH, THT).transpose(1, 2, 0, 3)
        ).astype(f8)
        in_maps.append({
            "xT": xTc, "x8": x8c,
            "Wt": np.ascontiguousarray(Wt[rot]),
            "W8": np.ascontiguousarray(W8[rot]),
            "be": np.ascontiguousarray(be[rot]),
            "WrT": np.ascontiguousarray(WrT[:, :, rot]),
            "brT": np.ascontiguousarray(brT[rot]),
        })
    return in_maps


def finalize(res):
    """Gather per-core fp16 outputs, un-scale, return [B, S, D] fp32."""
    out = np.concatenate(
        [np.asarray(res.results[c]["out"]).astype(np.float32) for c in range(N_CORES)],
        axis=0,
    )
    out *= 1.0 / SW
    return out.reshape(B, S, D)


_BUILT = {}


def get_built():
    if "nc" not in _BUILT:
        _BUILT["nc"] = build()
    return _BUILT["nc"]


def wait_device_ready(max_tries=8, sleep_s=20):
    """Poke the axon-tunneled devices until they respond. A crashed prior
    process can leave the remote exec unit wedged for a minute or two;
    the terminal recycles it on subsequent connection attempts."""
    import time

    import jax
    import jax.numpy as jnp

    for attempt in range(max_tries):
        try:
            devs = jax.devices()
            for d in devs[:1]:
                a = jax.device_put(jnp.ones((2, 2)), d)
                np.asarray(a)
            return True
        except Exception as exc:  # noqa: BLE001
            if attempt == max_tries - 1:
                raise
            print(f"device not ready (attempt {attempt + 1}): {exc}; retrying")
            time.sleep(sleep_s)
    return False


def run_spmd(in_maps, **kwargs):
    nc = get_built()
    wait_device_ready()
    try:
        return run_bass_kernel_spmd(
            nc, in_maps, core_ids=list(range(N_CORES)), **kwargs
        )
    except Exception as exc:  # noqa: BLE001
        print(f"run_bass_kernel_spmd failed ({exc}); retrying once after re-poke")
        wait_device_ready()
        return run_bass_kernel_spmd(
            nc, in_maps, core_ids=list(range(N_CORES)), **kwargs
        )


def kernel(x, W_experts, b_experts, W_router, b_router):
    in_maps = prep_inputs(x, W_experts, b_experts, W_router, b_router)
    res = run_spmd(in_maps)
    return finalize(res)


# revision 22
# speedup vs baseline: 1.0385x; 1.0026x over previous
"""MoE ExpertLayer kernel for Trainium2 (8 NeuronCores, data-parallel over tokens).

Reference computation (B=4, S=2048, D=1024, E=8):
    logits  = x @ W_router.T + b_router          # [B,S,E]
    probs   = softmax(logits, axis=-1)
    y_e     = x @ W_experts[e].T + b_experts[e]  # all experts, dense
    out     = sum_e probs[..., e] * y_e          # [B,S,D]

Sharding: data-parallel over the flattened token axis (8192 tokens -> 1024
tokens per core). Every core receives the full (transposed) expert weights and
computes its token shard end-to-end; no collectives are needed.

Per-core dataflow (two-phase; microbenched DR/bf16 matmul stream = 216ns per
N=512 instruction regardless of fp8/bf16 mode mixing, so the schedule aims at
an uninterrupted PE instruction stream):
  - Hybrid precision: per (expert, token-tile, col-half) the K=1024
    contraction runs as 1 fp8e4m3 DoubleRow matmul (first 256 of K, full 2x
    rate) + 6 bf16 matmuls, all accumulating in one fp32 PSUM group. 14
    instructions per (e, tt) instead of 16 all-bf16. Scale-relative absmax
    error ~1.85e-2 (gate 2e-2), dominated by fp8 quantization of x and W;
    bit-stable across runs (deterministic schedule + fixed-seed inputs).
  - Expert weights/biases are pre-scaled by 256 on the host so fp8 weights
    sit in e4m3's normal range; the host divides the output by 256 after.
  - Phase R(th): router logits for one token half accumulate in the first 8
    partitions of a rotating PSUM bank as the x dt-tiles arrive from DMA;
    z = Exp(logits + b_router) on the ACT copy out of PSUM (no max
    subtraction: |logits| <= ~2.6). z transposes token-major via DVE 32x32
    block transposes; probs = z * (1/sum z) via DVE reduce/reciprocal. The
    expert-bias fold acc[t,f] = sum_e z[t,e] b_e[f] / Z is a K=8 matmul with
    z.T stationary; the 1/Z normalization rides the PSUM->acc copy on the
    otherwise-idle ACT engine.
  - Phase E(e, tts): per token tile, [DR pe0, DR pe1, 6x (bf16 pe0, bf16
    pe1)] back-to-back; the combine acc += psum * probs[:,e] is one fused
    DVE scalar_tensor_tensor per half-tile. PSUM: pe0/pe1 x3 bufs + bias x2
    = 8 banks; triple buffering keeps group-opens ahead of combine drains.
  - Order: short warmup (8x N=256, PE p-state ramp during the
    preamble-to-first-data window); DR groups for tt0-2 open first (they
    need only x8+W8[0], ~384KB); R(th0); E(0, tt0-3) with bias folds after
    the w-gated bf16 streams; R(th1) runs one block early (during tt3) so
    its softmax tail + bias ACT copies drain before the th1 blocks; then
    E(1..7, all tts) as pure 14-instruction streams. Each core processes
    experts in a rotated order (host rotates the expert axis of
    Wt/W8/be/WrT/brT by core id) so the 8 cores' head-phase weight reads
    spread across different HBM regions instead of all pulling W[0] at
    once. The very last block closes/drains its pe0 half before streaming
    pe1, shortening the end-of-kernel combine+store chain.
  - Head DMA: every engine queue runs a ~7us preamble ending in an
    all-engine barrier before the first HWDGE trigger can issue, and
    triggers cost ~0.6us of queue time each (ring B = the ACT queue, so its
    up-front batch is kept short and th1/expert-1 triggers are emitted
    mid-program). Data flows from ~8.3us at ~300GB/s aggregate (8 cores
    share HBM), so expert 0's operands (x th0 1MB + W[0] 1.75MB) pace the
    head; ring order matches PE need-times. Tiny tensors are padded
    (brT [E,16]) and placed between bulk streams -- sub-64B packets
    otherwise stall a ring ~0.7us each. Experts 1..7 prefetch one ~0.3MB
    weight chunk per token-tile block (spread, not burst, to keep SBUF
    write pressure off the PE read path).
  - The final expert writes fp16 half-tiles which stream to DRAM as they
    finish (fp16 rounding invisible at this error scale). Fixed costs: ~7us
    preamble + ~4.5us end-of-NEFF barrier cascade; measured run-to-run
    variance ~+-2us from HBM/DMA contention.
"""

import os
import sys

for _p in ("/opt/trn_rl_repo", "/root/.axon_site/_ro/trn_rl_repo"):
    if os.path.isdir(_p) and _p not in sys.path:
        sys.path.insert(0, _p)

from contextlib import ExitStack

import ml_dtypes
import numpy as np

import concourse.bass as bass
import concourse.mybir as mybir
import concourse.tile as tile
from concourse import bacc
from concourse.bass import ts
from concourse.bass_utils import run_bass_kernel_spmd

B, S, D, E = 4, 2048, 1024, 8
N_CORES = 8
T = B * S // N_CORES  # tokens per core = 1024
P = 128               # partitions
TT = T // P           # token tiles per core = 8
DT = D // P           # contraction tiles = 8
FN = 512              # matmul moving free dim (one PSUM bank of fp32)
FH = D // FN          # output column halves = 2
TH = 2                # token halves per core
THT = T // TH         # 512 tokens per half

KP = 2                # fp8 k-pair (2 x 128 = 256 of K) per DR matmul
BFT = DT - KP         # bf16 k-tiles = 6
SW = 256.0            # host-side expert weight/bias scale


def build():
    """Build the per-core Bass/Tile program (identical SPMD program on all cores)."""
    bf16 = mybir.dt.bfloat16
    f8 = mybir.dt.float8e4
    f16 = mybir.dt.float16
    f32 = mybir.dt.float32
    DR = mybir.MatmulPerfMode.DoubleRow

    nc = bacc.Bacc("TRN2", target_bir_lowering=False, debug=False)

    xT_d = nc.dram_tensor("xT", [P, TH, DT, THT], bf16, kind="ExternalInput").ap()
    x8_d = nc.dram_tensor("x8", [P, TH, KP, THT], f8, kind="ExternalInput").ap()
    Wt_d = nc.dram_tensor("Wt", [E, P, BFT, D], bf16, kind="ExternalInput").ap()
    W8_d = nc.dram_tensor("W8", [E, P, KP, D], f8, kind="ExternalInput").ap()
    be_d = nc.dram_tensor("be", [E, D], bf16, kind="ExternalInput").ap()
    WrT_d = nc.dram_tensor("WrT", [P, DT, E], bf16, kind="ExternalInput").ap()
    brT_d = nc.dram_tensor("brT", [E, 16], f32, kind="ExternalInput").ap()
    out_d = nc.dram_tensor("out", [T, D], f16, kind="ExternalOutput").ap()

    with tile.TileContext(nc) as tc, ExitStack() as ctx:
        singles = ctx.enter_context(tc.tile_pool(name="singles", bufs=1))
        wpool = ctx.enter_context(tc.tile_pool(name="wpool", bufs=4))
        w8pool = ctx.enter_context(tc.tile_pool(name="w8pool", bufs=4))
        small = ctx.enter_context(tc.tile_pool(name="small", bufs=4))
        opool = ctx.enter_context(tc.tile_pool(name="opool", bufs=4))
        ppool = ctx.enter_context(tc.tile_pool(name="psum_e", bufs=3, space="PSUM"))
        pbias = ctx.enter_context(tc.tile_pool(name="psum_b", bufs=2, space="PSUM"))

        rA, rB = nc.sync, nc.scalar  # the two HWDGE rings

        WrT = singles.tile([P, DT, E], bf16)
        brT = singles.tile([E, 16], f32)
        be = singles.tile([E, D], bf16)
        xT = singles.tile([P, TH, DT, THT], bf16)
        x8 = singles.tile([P, TH, KP, THT], f8)
        w8_0 = w8pool.tile([P, KP, D], f8, tag="w8")
        w0 = wpool.tile([P, BFT, D], bf16, tag="w")

        # ---- Head DMA schedule (issue order per ring == arrival order;
        # trigger instructions cost ~0.6us on their host queue, and ring B =
        # the ACT queue, so B's up-front trigger batch is kept short — the
        # th1/expert-1 triggers are emitted mid-program, after the router-th0
        # Exp, so they don't delay it).
        # Order matches PE need-times: router x first, then DR operands,
        # then W[0] k-chunks; tiny gating tensors ride between bulk streams.
        rA.dma_start(out=x8[:, 0], in_=x8_d[:, 0])
        rA.dma_start(out=xT[:, 0, 0:2], in_=xT_d[:, 0, 0:2])
        rA.dma_start(out=xT[:, 0, 4:6], in_=xT_d[:, 0, 4:6])
        rA.dma_start(out=w0[:, 0:1], in_=Wt_d[0, :, 0:1])
        rA.dma_start(out=w0[:, 2:3], in_=Wt_d[0, :, 2:3])
        rA.dma_start(out=w0[:, 4:5], in_=Wt_d[0, :, 4:5])
        rB.dma_start(out=w8_0, in_=W8_d[0])
        rB.dma_start(out=WrT, in_=WrT_d)
        rB.dma_start(out=xT[:, 0, 2:4], in_=xT_d[:, 0, 2:4])
        rB.dma_start(out=xT[:, 0, 6:8], in_=xT_d[:, 0, 6:8])
        rB.dma_start(out=brT, in_=brT_d)
        rB.dma_start(out=w0[:, 1:2], in_=Wt_d[0, :, 1:2])
        rB.dma_start(out=w0[:, 3:4], in_=Wt_d[0, :, 3:4])
        rB.dma_start(out=w0[:, 5:6], in_=Wt_d[0, :, 5:6])
        rB.dma_start(out=be, in_=be_d)

        # ---- A short PE warm-up over a memset tile starts the p-state ramp
        # during the preamble-to-first-data window (~7-8.6us).
        scratch = singles.tile([P, FN], bf16)
        nc.gpsimd.memset(scratch, 0.0)
        pwarm = ppool.tile([P, FN], f32, tag="pe0")
        for _ in range(8):
            nc.tensor.matmul(
                pwarm[:, :256], scratch[:, :P], scratch[:, :256],
                start=True, stop=True,
            )

        acc = singles.tile([P, TT, D], f32)
        probs = singles.tile([P, TT, E], f32)
        # z lives on 32 partitions (zero-padded past E) so the [8,THT]->[tok,8]
        # transpose runs as DVE 32x32 block transposes (no PSUM/PE time).
        zTp = singles.tile([32, TH, THT], f32)
        nc.vector.memset(zTp, 0.0)
        zTb = singles.tile([E, TH, THT], bf16)
        pTs = singles.tile([P, TT, 32], f32)

        out_dst = out_d.rearrange("(tt p) f -> p tt f", p=P)
        wrings = [rA, rB]

        def router_logits(th):
            # logits accumulate in the first 8 partitions of a pbias-ring
            # bank (frees the expert pe0 ring so three DR groups can open
            # before the router runs)
            prt = pbias.tile([P, FN], f32, tag="pb")
            pr = prt[:E, :]
            for dt_ in range(DT):
                nc.tensor.matmul(
                    pr, WrT[:, dt_, :], xT[:, th, dt_, :],
                    start=(dt_ == 0), stop=(dt_ == DT - 1),
                )
            nc.scalar.activation(
                out=zTp[:E, th], in_=pr,
                func=mybir.ActivationFunctionType.Exp, bias=brT[:, 0:1], scale=1.0,
            )
            nc.vector.tensor_copy(zTb[:, th], zTp[:E, th])

        recs = {}

        def router_probs(tt):
            # DVE-only softmax tail: transpose z token-major, normalize.
            th = tt // (TT // TH)
            ti = tt % (TT // TH)
            for j in range(P // 32):
                nc.vector.transpose(
                    out=pTs[j * 32 : (j + 1) * 32, tt, :],
                    in_=zTp[:, th, ti * P + j * 32 : ti * P + (j + 1) * 32],
                )
            pT = pTs[:, tt, :E]
            ssum = small.tile([P, 1], f32, tag="ssum")
            nc.vector.reduce_sum(out=ssum, in_=pT, axis=mybir.AxisListType.X)
            rec = small.tile([P, 1], f32, tag="rec")
            nc.vector.reciprocal(rec, ssum)
            nc.vector.tensor_scalar_mul(probs[:, tt, :], pT, rec)
            recs[tt] = rec

        def bias_fold(tt):
            # acc[t, f] = sum_e z[t, e] * b_e[f] / sum(z): K=8 matmul with
            # z.T stationary; the 1/Z normalization rides the PSUM->acc copy
            # on the ACT engine. Must be emitted before the combines of this
            # tt (they read-modify acc).
            th = tt // (TT // TH)
            tok = ts(tt % (TT // TH), P)
            rec = recs.pop(tt)
            for fh in range(FH):
                pb = pbias.tile([P, FN], f32, tag="pb")
                nc.tensor.matmul(
                    pb, zTb[:, th, tok], be[:, ts(fh, FN)],
                    start=True, stop=True,
                )
                nc.scalar.activation(
                    out=acc[:, tt, ts(fh, FN)], in_=pb,
                    func=mybir.ActivationFunctionType.Identity, scale=rec,
                )

        def expert_dr(w8, tt):
            # open the (pe0, pe1) PSUM group for tt with the fp8 DR pair
            th = tt // (TT // TH)
            tok = ts(tt % (TT // TH), P)
            pe0 = ppool.tile([P, FN], f32, tag="pe0")
            pe1 = ppool.tile([P, FN], f32, tag="pe1")
            lhs8 = x8[:, th, :, tok]
            nc.tensor.matmul(
                pe0, lhs8, w8[:, :, 0:FN], start=True, stop=False,
                perf_mode=mybir.MatmulPerfMode.DoubleRow,
            )
            nc.tensor.matmul(
                pe1, lhs8, w8[:, :, FN : 2 * FN], start=True, stop=False,
                perf_mode=mybir.MatmulPerfMode.DoubleRow,
            )
            return pe0, pe1

        def expert_bf16(w, tt, pes):
            pe0, pe1 = pes
            th = tt // (TT // TH)
            tok = ts(tt % (TT // TH), P)
            for dt_ in range(BFT):
                lhsT = xT[:, th, KP + dt_, tok]
                sp = dt_ == BFT - 1
                nc.tensor.matmul(pe0, lhsT, w[:, dt_, 0:FN], start=False, stop=sp)
                nc.tensor.matmul(
                    pe1, lhsT, w[:, dt_, FN : 2 * FN], start=False, stop=sp
                )

        def expert_combine(e, tt, pes, fhs=(0, 1)):
            for fh, pe_ in ((0, pes[0]), (1, pes[1])):
                if fh not in fhs:
                    continue
                if e == E - 1:
                    # final expert: write the finished fp16 half-tile and
                    # stream it out so stores overlap remaining compute
                    o16 = opool.tile([P, FN], f16, tag="o16")
                    nc.vector.scalar_tensor_tensor(
                        out=o16, in0=pe_, scalar=probs[:, tt, e : e + 1],
                        in1=acc[:, tt, ts(fh, FN)],
                        op0=mybir.AluOpType.mult, op1=mybir.AluOpType.add,
                    )
                    wrings[fh].dma_start(out=out_dst[:, tt, ts(fh, FN)], in_=o16)
                else:
                    # acc = psum * probs[:, e] + acc  (one fused DVE op)
                    nc.vector.scalar_tensor_tensor(
                        out=acc[:, tt, ts(fh, FN)], in0=pe_,
                        scalar=probs[:, tt, e : e + 1],
                        in1=acc[:, tt, ts(fh, FN)],
                        op0=mybir.AluOpType.mult, op1=mybir.AluOpType.add,
                    )

        def expert_block(e, w, w8, tts, pf=None):
            for tt in tts:
                if pf is not None and tt < len(pf):
                    pf[tt]()
                pes = expert_dr(w8, tt)
                if e == E - 1 and tt == TT - 1:
                    # final block: close/drain the pe0 half first so its
                    # combine+store overlap the pe1 stream (shorter tail)
                    th = tt // (TT // TH)
                    tok = ts(tt % (TT // TH), P)
                    for dt_ in range(BFT):
                        nc.tensor.matmul(
                            pes[0], xT[:, th, KP + dt_, tok], w[:, dt_, 0:FN],
                            start=False, stop=(dt_ == BFT - 1),
                        )
                    expert_combine(e, tt, pes, fhs=(0,))
                    for dt_ in range(BFT):
                        nc.tensor.matmul(
                            pes[1], xT[:, th, KP + dt_, tok],
                            w[:, dt_, FN : 2 * FN],
                            start=False, stop=(dt_ == BFT - 1),
                        )
                    expert_combine(e, tt, pes, fhs=(1,))
                else:
                    expert_bf16(w, tt, pes)
                    expert_combine(e, tt, pes)

        def prefetch_actions(e, w, w8):
            # one ~0.3MB weight chunk per token-tile block: spreads the DMA
            # bursts so SBUF write pressure doesn't stall the PE read path
            acts = [lambda w8=w8, e=e: rA.dma_start(out=w8, in_=W8_d[e])]
            for k in range(BFT):
                r = rA if k % 2 == 0 else rB
                acts.append(
                    lambda w=w, e=e, k=k, r=r: r.dma_start(
                        out=w[:, k : k + 1], in_=Wt_d[e, :, k : k + 1]
                    )
                )
            return acts

        # ---- Phase order per half: R(th); DR-open tt/tt+1 (x8/w8 land
        # first); their bf16 streams (w k-chunk paced); bias folds (by then
        # the ACT Exp + DVE softmax tail are long done -> no PE stall);
        # combines; then the remaining two tts as full blocks. Experts 1..7
        # are pure 14-instruction streams.
        p0 = expert_dr(w8_0, 0)
        p1 = expert_dr(w8_0, 1)
        p2 = expert_dr(w8_0, 2)
        router_logits(0)
        for tt in range(0, 4):
            router_probs(tt)
        # th1 / expert-1 loads, emitted here so ring B's up-front trigger
        # batch stays short (ring B shares the ACT queue with the Exp above).
        rA.dma_start(out=xT[:, 1, 0:4], in_=xT_d[:, 1, 0:4])
        rA.dma_start(out=xT[:, 1, 4:8], in_=xT_d[:, 1, 4:8])
        rB.dma_start(out=x8[:, 1], in_=x8_d[:, 1])
        w1 = wpool.tile([P, BFT, D], bf16, tag="w")
        w8_1 = w8pool.tile([P, KP, D], f8, tag="w8")
        rA.dma_start(out=w8_1, in_=W8_d[1])
        rA.dma_start(out=w1[:, 0:3, :], in_=Wt_d[1, :, 0:3, :])
        rB.dma_start(out=w1[:, 3:6, :], in_=Wt_d[1, :, 3:6, :])
        expert_bf16(w0, 0, p0)
        bias_fold(0)
        expert_combine(0, 0, p0)
        expert_bf16(w0, 1, p1)
        bias_fold(1)
        expert_combine(0, 1, p1)
        p3 = expert_dr(w8_0, 3)
        expert_bf16(w0, 2, p2)
        bias_fold(2)
        expert_combine(0, 2, p2)
        # R(th1) here: x th1 has arrived by now, and running it an expert
        # block early lets the softmax tail + bias ACT copies drain during
        # tt3 instead of stalling the th1 blocks and expert 1's start
        router_logits(1)
        for tt in range(4, TT):
            router_probs(tt)
        expert_bf16(w0, 3, p3)
        bias_fold(3)
        expert_combine(0, 3, p3)
        for tt in range(4, TT):
            pes = expert_dr(w8_0, tt)
            bias_fold(tt)
            expert_bf16(w0, tt, pes)
            expert_combine(0, tt, pes)

        ws = {1: (w1, w8_1)}
        for e in range(1, E):
            pf = None
            if e + 1 < E:
                wn = wpool.tile([P, BFT, D], bf16, tag="w")
                w8n = w8pool.tile([P, KP, D], f8, tag="w8")
                ws[e + 1] = (wn, w8n)
                pf = prefetch_actions(e + 1, wn, w8n)
            w, w8 = ws.pop(e)
            expert_block(e, w, w8, range(TT), pf=pf)

    nc.compile()
    return nc


def prep_inputs(x, W_experts, b_experts, W_router, b_router):
    """Host-side marshalling: shard tokens, transpose so the contraction dim
    is DMA-contiguous onto SBUF partitions, cast/scale to compute dtypes."""
    bf = ml_dtypes.bfloat16
    f8 = ml_dtypes.float8_e4m3fn
    x = np.asarray(x, dtype=np.float32).reshape(B * S, D)
    WeT = np.asarray(W_experts, dtype=np.float32).transpose(0, 2, 1) * SW
    # bf16 k-tiles KP..DT: [E, D_in, D_out] -> [E, P, BFT, D_out]
    Wt = np.ascontiguousarray(
        WeT[:, KP * P :, :].reshape(E, BFT, P, D).transpose(0, 2, 1, 3)
    ).astype(bf)
    W8 = np.ascontiguousarray(
        WeT[:, : KP * P, :].reshape(E, KP, P, D).transpose(0, 2, 1, 3)
    ).astype(f8)
    WrT = np.ascontiguousarray(
        np.asarray(W_router, dtype=np.float32).T.reshape(DT, P, E).transpose(1, 0, 2)
    ).astype(bf)
    be = (np.asarray(b_experts, dtype=np.float32) * SW).astype(bf)
    brT = np.zeros((E, 16), np.float32)
    brT[:, 0] = np.asarray(b_router, dtype=np.float32)
    in_maps = []
    for c in range(N_CORES):
        # rotate the expert axis per core so the 8 cores' head-phase weight
        # reads hit different HBM regions instead of all pulling W[0] at
        # once; the math is order-invariant (probs/bias rotate with it)
        rot = [(j + c) % E for j in range(E)]
        xs = x[c * T : (c + 1) * T, :].T  # [D, T]
        xTc = np.ascontiguousarray(
            xs.reshape(DT, P, TH, THT).transpose(1, 2, 0, 3)  # [P, TH, DT, THT]
        ).astype(bf)
        x8c = np.ascontiguousarray(
            xs[: KP * P].reshape(KP, P, TH, THT).transpose(1, 2, 0, 3)
        ).astype(f8)
        in_maps.append({
            "xT": xTc, "x8": x8c,
            "Wt": np.ascontiguousarray(Wt[rot]),
            "W8": np.ascontiguousarray(W8[rot]),
            "be": np.ascontiguousarray(be[rot]),
            "WrT": np.ascontiguousarray(WrT[:, :, rot]),
            "brT": np.ascontiguousarray(brT[rot]),
        })
    return in_maps


def finalize(res):
    """Gather per-core fp16 outputs, un-scale, return [B, S, D] fp32."""
    out = np.concatenate(
        [np.asarray(res.results[c]["out"]).astype(np.float32) for c in range(N_CORES)],
        axis=0,
    )
    out *= 1.0 / SW
    return out.reshape(B, S, D)


_BUILT = {}


def get_built():
    if "nc" not in _BUILT:
        _BUILT["nc"] = build()
    return _BUILT["nc"]


def wait_device_ready(max_tries=8, sleep_s=20):
    """Poke the axon-tunneled devices until they respond. A crashed prior
    process can leave the remote exec unit wedged for a minute or two;
    the terminal recycles it on subsequent connection attempts."""
    import time

    import jax
    import jax.numpy as jnp

    for attempt in range(max_tries):
        try:
            devs = jax.devices()
            for d in devs[:1]:
                a = jax.device_put(jnp.ones((2, 2)), d)
                np.asarray(a)
            return True
        except Exception as exc:  # noqa: BLE001
            if attempt == max_tries - 1:
                raise
            print(f"device not ready (attempt {attempt + 1}): {exc}; retrying")
            time.sleep(sleep_s)
    return False


def run_spmd(in_maps, **kwargs):
    nc = get_built()
    wait_device_ready()
    try:
        return run_bass_kernel_spmd(
            nc, in_maps, core_ids=list(range(N_CORES)), **kwargs
        )
    except Exception as exc:  # noqa: BLE001
        print(f"run_bass_kernel_spmd failed ({exc}); retrying once after re-poke")
        wait_device_ready()
        return run_bass_kernel_spmd(
            nc, in_maps, core_ids=list(range(N_CORES)), **kwargs
        )


def kernel(x, W_experts, b_experts, W_router, b_router):
    in_maps = prep_inputs(x, W_experts, b_experts, W_router, b_router)
    res = run_spmd(in_maps)
    return finalize(res)
